# revision 108
# baseline (speedup 1.0000x reference)
"""Multi-head self-attention (B=4, T=2048, E=1024, H=16) on 8 trn2 NeuronCores.

Sharding: core (b, h) = batch b, token-half h. Each core computes K/V for the
full sequence (duplicated within the batch pair), Q for its own 8 query blocks
of 128 tokens, causal attention for those blocks, then the output projection
and LayerNorm for its own tokens.

Attention restructure (vs the 128-wide-per-head-pair baseline): each core's
query blocks are ordered by DESCENDING padded causal length (16,14,12,10 |
8,6,4,2 key blocks), so for key block j the active query blocks form a
contiguous prefix. Scores/AV run one matmul per (head, group-of-4-q-blocks,
key block) with free dim up to 512, cutting PE instruction count ~3x. The
softmax denominator division runs once per (head, group) on 512 columns.
Projection+LN for the first 4 token blocks is interleaved into the second
(light) attention group to shrink the tail.

Causal balance: query blocks are paired (j, 15-j) so both cores of a batch
process blocks with padded key-lengths 2,4,...,16; host-supplied mask tiles
encode the true causal structure, keeping the compiled program identical
across cores (SPMD).

All matmuls run in bf16 with fp32 PSUM accumulation (validated ~2e-3
scale-relative error vs the fp32 reference).
"""
import json
import numpy as np
import ml_dtypes
from contextlib import ExitStack

import concourse.bass as bass
import concourse.bass_utils as _bass_utils
import concourse.tile as tile
from concourse import mybir
from concourse.bass_utils import run_bass_kernel_spmd

# ----------------------------------------------------------------------------
# Toolchain workarounds for this container's walrus build (see birfix notes):
# 1. EVENT_SEMAPHORE_RANGE_CLEAR InstISA is rejected ("ISA wrong length").
# 2. Engine instructions only carry one semaphore-wait slot; extra waits are
#    peeled onto NoOp carriers on the same engine (order-preserving).
# ----------------------------------------------------------------------------


def _patched_clear_and_free_semaphores(self, sems):
    if not sems:
        return
    sem_nums = [s.num if hasattr(s, "num") else s for s in sems]
    self._state.prepend_free_semaphores(sem_nums)
    for poison_set in self._tile_sem_poison_stack:
        poison_set.update(sem_nums)


def _fix_bir_waits(bir_json: bytes) -> bytes:
    bir = json.loads(bir_json)
    ctr = 0
    changed = False
    for func in bir.get("functions", []):
        for blk in func.get("blocks", []):
            out = []
            for inst in blk.get("instructions", []):
                si = inst.get("sync_info") or {}
                waits = si.get("on_wait") or []
                if len(waits) > 1:
                    for w in waits[:-1]:
                        ctr += 1
                        out.append(
                            {
                                "debug": inst.get("debug"),
                                "engine": inst.get("engine", "SP"),
                                "ins": [],
                                "name": f"IWF-{ctr}",
                                "opcode": "NoOp",
                                "outs": [],
                                "sync_info": {"on_wait": [w]},
                            }
                        )
                    si = dict(si)
                    si["on_wait"] = waits[-1:]
                    inst = dict(inst)
                    inst["sync_info"] = si
                    changed = True
                out.append(inst)
            blk["instructions"] = out
    return json.dumps(bir).encode() if changed else bir_json


_orig_compile_bir_kernel = _bass_utils.compile_bir_kernel


def _patched_compile_bir_kernel(bir_json, tmpdir, neff_name="file.neff"):
    if isinstance(bir_json, str):
        bir_json = bir_json.encode()
    return _orig_compile_bir_kernel(_fix_bir_waits(bir_json), tmpdir, neff_name)


def _install_patches():
    if getattr(bass.Bass, "_mhsa_patched", False):
        return
    bass.Bass.clear_and_free_semaphores = _patched_clear_and_free_semaphores
    bass.Bass._mhsa_patched = True
    _bass_utils.compile_bir_kernel = _patched_compile_bir_kernel
    try:
        import concourse.bass2jax as _b2j

        _b2j.compile_bir_kernel = _patched_compile_bir_kernel
    except ImportError:
        pass


_install_patches()

# ----------------------------------------------------------------------------
# Problem constants (hardcoded per spec)
# ----------------------------------------------------------------------------
B, T, E, H = 4, 2048, 1024, 16
HD = E // H  # 64
P = 128
NB = T // P  # 16 query/key blocks
NQ = 8  # query blocks per core
EC = E // P  # 8 e-chunks
SCALE = 1.0 / float(np.sqrt(T))
EPS = 1e-6
BF = mybir.dt.bfloat16
F32 = mybir.dt.float32
F8 = mybir.dt.float8e4
NPBF = ml_dtypes.bfloat16
NPF8 = ml_dtypes.float8_e4m3
# fp8 weights are pre-scaled by 16 host-side (keeps N(0, 0.02) entries out
# of the fp8e4m3 subnormal range); the PSUM drain multiplies by 1/16
WSCALE = 16.0
WINV = 1.0 / WSCALE
DR = mybir.MatmulPerfMode.DoubleRow

# query-block assignment: pairs (j, 15-j) so both cores of a batch pair see
# padded lengths {2,4,...,16}; blocks listed in ASCENDING padded length
BLOCKS_A = [0, 2, 4, 6, 9, 11, 13, 15]  # true lengths 1,3,5,7,10,12,14,16
BLOCKS_B = [1, 3, 5, 7, 8, 10, 12, 14]  # true lengths 2,4,6,8,9,11,13,15

# device-side q-block order: DESCENDING padded length; two groups of 4
GLS = {0: (16, 14, 12, 10), 1: (8, 6, 4, 2)}
# mask instances: (group, key block j, boundary q-block index bi); the
# boundary block is always the LAST active block of the prefix at that j
MASK_INST = []
for _g in (0, 1):
    for _j in range(GLS[_g][0]):
        for _bi, _L in enumerate(GLS[_g]):
            if _j in (_L - 2, _L - 1):
                MASK_INST.append((_g, _j, _bi))
MASK_IDX = {(g, j): (idx, bi) for idx, (g, j, bi) in enumerate(MASK_INST)}
assert len(MASK_INST) == 16


def _width(g, j):
    return 128 * sum(1 for L in GLS[g] if L > j)


_nc_cache = {}


def _build_nc():
    if "nc" in _nc_cache:
        return _nc_cache["nc"]
    nc = bass.Bass(num_devices=8)

    # inputs (per-core)
    xT_d = nc.dram_tensor("xT", [E, T], F8, kind="ExternalInput")
    xTq_d = nc.dram_tensor("xTq", [E, NQ * P], BF, kind="ExternalInput")
    xT8q_d = nc.dram_tensor("xT8q", [E, NQ * P], F8, kind="ExternalInput")
    WqT_d = nc.dram_tensor("WqT", [E, E], F8, kind="ExternalInput")
    WkT_d = nc.dram_tensor("WkT", [E, E], F8, kind="ExternalInput")
    WvT_d = nc.dram_tensor("WvT", [E, E], F8, kind="ExternalInput")
    WpT_d = nc.dram_tensor("WpT", [E, E], BF, kind="ExternalInput")
    cF32_d = nc.dram_tensor("cF32", [P, 16], F32, kind="ExternalInput")
    cBF_d = nc.dram_tensor("cBF", [P, 4 * E + 16 * P], BF, kind="ExternalInput")
    y_d = nc.dram_tensor("y", [NQ, P, E], BF, kind="ExternalOutput")

    with tile.TileContext(nc) as tc:
        with ExitStack() as ctx:
            consts = ctx.enter_context(tc.tile_pool(name="consts", bufs=1))
            big = ctx.enter_context(tc.tile_pool(name="big", bufs=1))
            wpool = ctx.enter_context(tc.tile_pool(name="wpool", bufs=1))
            # xT is only needed during the QKV phase; its pool is closed
            # before the attention working set is allocated
            xtp = ctx.enter_context(tc.tile_pool(name="xtp", bufs=1))
            _psA_cm = tc.tile_pool(name="psA", bufs=1, space="PSUM")
            ps = _psA_cm.__enter__()

            def load_w(dram, name, interleave_with=None, split_first=False,
                       dtype=F8):
                # two half-tiles in a 3-slot rotation: the next projection's
                # first half streams in while the previous one's second half
                # is still being consumed. ONE DMA per half (HWDGE issue is a
                # serialized ~625ns/DMA shared resource — minimize count)
                halves = []
                for hf in range(2):
                    w = wpool.tile(
                        [P, EC, E // 2], dtype, tag="wh", bufs=3,
                        name=f"{name}{hf}"
                    )
                    if split_first and hf == 0:
                        # first fb-chunk, low c-half first: the first matmul
                        # group's opening DoubleRow pairs start sooner
                        nc.sync.dma_start(
                            w[:, 0:4, 0:P],
                            dram.rearrange("(c p) f -> p c f", p=P)[:, 0:4, 0:P],
                        )
                        nc.sync.dma_start(
                            w[:, 4:EC, 0:P],
                            dram.rearrange("(c p) f -> p c f", p=P)[:, 4:EC, 0:P],
                        )
                        if interleave_with is not None:
                            interleave_with(hf)
                        nc.sync.dma_start(
                            w[:, :, P:512],
                            dram.rearrange("(c p) f -> p c f", p=P)[:, :, P:512],
                        )
                    else:
                        nc.sync.dma_start(
                            w[:, :, :],
                            dram.rearrange("(c p) f -> p c f", p=P)[
                                :, :, hf * 512 : (hf + 1) * 512
                            ],
                        )
                        if interleave_with is not None:
                            interleave_with(hf)
                    halves.append(w)
                return halves

            # PE-critical loads first. HWDGE queue order: Wk half0, xT win0,
            # f32 consts (bk needed by the first bias add), Wk half1, then
            # the remaining xT windows — the ts-outer K loop consumes one
            # window per ~13.7us so the serialized DMA stream stays ahead
            xT = xtp.tile([P, EC, T], F8)
            cF32 = consts.tile([P, 16], F32)
            cBF = consts.tile([P, 4 * E + 16 * P], BF)

            def _xt_w(wi):
                if wi == 0:
                    # window 0 in c-halves: the first DoubleRow matmuls only
                    # need c-chunks 0-3, so the startup chain shortens ~1us
                    for ch in (slice(0, 4), slice(4, EC)):
                        nc.sync.dma_start(
                            xT[:, ch, 0:512],
                            xT_d.rearrange("(c p) t -> p c t", p=P)[
                                :, ch, 0:512
                            ],
                        )
                    return
                nc.sync.dma_start(
                    xT[:, :, wi * 512 : (wi + 1) * 512],
                    xT_d.rearrange("(c p) t -> p c t", p=P)[
                        :, :, wi * 512 : (wi + 1) * 512
                    ],
                )

            def _wk_companion(hf):
                if hf == 0:
                    _xt_w(0)
                    nc.sync.dma_start(cF32[:, :], cF32_d[:, :])

            # serialized-DMA ordering: everything attention-start needs goes
            # first (xT8q for Q, cBF for masks, Wv); the big bf16 xTq is only
            # read by the residual ~150us in, so its DMA is issued after all
            # weight loads (tile allocated here, transfer deferred)
            Wk = load_w(WkT_d, "Wk", interleave_with=_wk_companion, split_first=True)
            _xt_w(1)
            _xt_w(2)
            _xt_w(3)
            xT8q = big.tile([P, EC, NQ * P], F8)
            nc.sync.dma_start(
                xT8q[:, :, :], xT8q_d.rearrange("(c p) t -> p c t", p=P)[:, :, :]
            )
            nc.sync.dma_start(cBF[:, :], cBF_d[:, :])
            xTq = big.tile([P, EC, NQ * P], BF)
            wvp = ctx.enter_context(tc.tile_pool(name="wvp", bufs=1))
            Wv = []
            for hf in range(2):
                wv = wvp.tile([P, EC, E // 2], F8, tag=f"wv{hf}", bufs=1,
                              name=f"Wv{hf}")
                nc.sync.dma_start(
                    wv[:, :, :],
                    WvT_d.rearrange("(c p) f -> p c f", p=P)[
                        :, :, hf * 512 : (hf + 1) * 512
                    ],
                )
                Wv.append(wv)
            # packed-constant layout in cBF: bv | bp | gamma | beta | masks
            OFF_BV, OFF_BP, OFF_G, OFF_B, OFF_M = 0, E, 2 * E, 3 * E, 4 * E

            def mall_at(idx):
                return cBF[:, OFF_M + idx * P : OFF_M + (idx + 1) * P]

            ones64 = consts.tile([P, 64], BF)
            nc.vector.memset(ones64[:], 1.0)

            # persistent intermediates
            KT = big.tile([P, EC, T], BF)  # K^T  [f, t]
            QT = big.tile([P, EC, NQ * P], BF)  # Q^T  [f, t_own]
            Vx = big.tile([P, NB, H, HD + 1], BF)  # V ext [t, h, d|1]
            zT = big.tile([P, EC, NQ * P], BF)  # z^T  [e, t_own]
            nc.vector.memset(Vx[:, :, :, HD : HD + 1], 1.0)

            # ---------------- K^T = Wk^T.T-chunks x xT + bk ----------------
            # ts outer: each xT window feeds all 8 fb groups (~13.7us of PE
            # work) so the next window's DMA completes in the shadow
            for ts_ in range(T // 512):
                for fb in range(EC):
                    pk = ps.tile([P, 512], F32, tag="mm512", bufs=6, name="pk")
                    for c in range(0, EC, 2):
                        nc.tensor.matmul(
                            pk[:],
                            Wk[fb // 4][:, c : c + 2, (fb % 4) * P : (fb % 4 + 1) * P],
                            xT[:, c : c + 2, ts_ * 512 : (ts_ + 1) * 512],
                            start=(c == 0),
                            stop=(c == EC - 2),
                            perf_mode=DR,
                        )
                    # drains alternate DVE/ACT (ACT is idle until attention;
                    # K^T partitions are features, so the bias is
                    # per-partition — exactly activation's bias operand).
                    # With fp8 the PE is 4x faster here and a single drain
                    # engine becomes the QKV-phase bottleneck.
                    if fb % 2 == 0:
                        nc.scalar.activation(
                            KT[:, fb, ts_ * 512 : (ts_ + 1) * 512],
                            pk[:],
                            mybir.ActivationFunctionType.Identity,
                            scale=WINV,
                            bias=cF32[:, fb : fb + 1],
                        )
                    else:
                        nc.vector.tensor_scalar(
                            out=KT[:, fb, ts_ * 512 : (ts_ + 1) * 512],
                            in0=pk[:],
                            scalar1=WINV,
                            scalar2=cF32[:, fb : fb + 1],
                            op0=mybir.AluOpType.mult,
                            op1=mybir.AluOpType.add,
                        )

            # ---------------- V (heads 0-7, blocks 0-7) ---------------------
            # V blocks 8-15, V heads 8-15, and Q feature-blocks 4-7 are
            # deferred: their matmul groups interleave into the exp-bound
            # attention wave, keeping PE busy while the Activation engine
            # catches up on exponentials. Wv lives in its own persistent pool
            # because its readers now extend deep into the attention phase
            # (the 3-slot weight rotation would reuse its slots too early).
            def emit_v_group(tb, fs, pool_tag):
                pv = ps.tile([P, 512], F32, tag=pool_tag, bufs=6 if pool_tag == "mm512" else 2, name="pv")
                for c in range(0, EC, 2):
                    nc.tensor.matmul(
                        pv[:],
                        xT[:, c : c + 2, tb * P : (tb + 1) * P],
                        Wv[fs][:, c : c + 2, :],
                        start=(c == 0),
                        stop=(c == EC - 2),
                        perf_mode=DR,
                    )
                # bv is NOT added here: softmax rows sum to 1, so the bias
                # contributes exactly bv per feature after normalization —
                # the host pre-adds it into the residual input xTq instead.
                # Scale-only drain; pre-phase drains alternate onto the
                # still-idle ACT engine.
                if pool_tag == "mm512" and tb % 2 == 0:
                    nc.scalar.activation(
                        Vx[:, tb, fs * 8 : (fs + 1) * 8, 0:HD],
                        pv[:, :].rearrange("p (h d) -> p h d", d=HD),
                        mybir.ActivationFunctionType.Identity,
                        scale=WINV,
                    )
                else:
                    nc.vector.tensor_scalar_mul(
                        Vx[:, tb, fs * 8 : (fs + 1) * 8, 0:HD],
                        pv[:, :].rearrange("p (h d) -> p h d", d=HD),
                        WINV,
                    )

            for tb in range(8):
                emit_v_group(tb, 0, "mm512")

            # ---------------- Q^T (fb 0-3) = Wq^T-chunks x xTq + bq --------
            Wq = load_w(WqT_d, "Wq")

            def emit_q_group(fb, ts_, pool_tag):
                pq = ps.tile([P, 512], F32, tag=pool_tag, bufs=6 if pool_tag == "mm512" else 2, name="pq")
                for c in range(0, EC, 2):
                    nc.tensor.matmul(
                        pq[:],
                        Wq[fb // 4][:, c : c + 2, (fb % 4) * P : (fb % 4 + 1) * P],
                        xT8q[:, c : c + 2, ts_ * 512 : (ts_ + 1) * 512],
                        start=(c == 0),
                        stop=(c == EC - 2),
                        perf_mode=DR,
                    )
                if pool_tag == "mm512" and fb % 2 == 0:
                    # pre-phase only: ACT is idle before attention starts
                    nc.scalar.activation(
                        QT[:, fb, ts_ * 512 : (ts_ + 1) * 512],
                        pq[:],
                        mybir.ActivationFunctionType.Identity,
                        scale=WINV,
                        bias=cF32[:, EC + fb : EC + fb + 1],
                    )
                else:
                    nc.vector.tensor_scalar(
                        out=QT[:, fb, ts_ * 512 : (ts_ + 1) * 512],
                        in0=pq[:],
                        scalar1=WINV,
                        scalar2=cF32[:, EC + fb : EC + fb + 1],
                        op0=mybir.AluOpType.mult,
                        op1=mybir.AluOpType.add,
                    )

            for fb in range(4):
                for ts_ in range(NQ * P // 512):
                    emit_q_group(fb, ts_, "mm512")

            # Wp streams in during attention (3-slot rotation frees Wq slots)
            Wp = load_w(WpT_d, "Wp", dtype=BF)
            # deferred residual input (first read at wave B part 2)
            nc.sync.dma_start(
                xTq[:, :, :], xTq_d.rearrange("(c p) t -> p c t", p=P)[:, :, :]
            )

            # ---------------- attention ----------------
            work = ctx.enter_context(tc.tile_pool(name="work", bufs=2))
            _psA_cm.__exit__(None, None, None)
            _psB_cm = tc.tile_pool(name="psB", bufs=1, space="PSUM")
            ps = _psB_cm.__enter__()

            # unit schedule: wave A = heads 0-7 (g0 then g1), wave B = heads
            # 8-15 g0, residual(0), heads 8-15 g1 with proj tb0-3 interleaved
            units = []
            for h in range(8):
                for g in (0, 1):
                    for p_ in range(GLS[g][0] // 2):
                        units.append((g, h, p_))
            for h in range(8, H):
                for p_ in range(GLS[0][0] // 2):
                    units.append((0, h, p_))
            for h in range(8, H):
                for p_ in range(GLS[1][0] // 2):
                    units.append((1, h, p_))
            WAVE_A_N = 96  # units in wave A
            PART2_AT = WAVE_A_N + 64  # first (g1, h>=8) unit: residual(0) here

            # fillers: V key-blocks 8-15 for heads 0-7 land up front (needed
            # by h0's AV from unit 4 on), then deferred V heads 8-15 and
            # Q fb 4-7 spread through wave A and slightly into wave B part 1;
            # projection tb0-3 spread through part 2
            fillers = {}
            for tb in range(8, NB):
                fillers.setdefault((tb - 8) // 2, []).append(("v0", tb))
            deferred = []
            for tb in range(NB):
                deferred.append(("v", tb))
                if tb < 8:
                    deferred.append(("q", tb))
            for i, d in enumerate(deferred):
                fillers.setdefault(
                    4 + i * (WAVE_A_N + 1) // len(deferred), []
                ).append(d)
            for i in range(4):
                fillers.setdefault(PART2_AT + 6 + 8 * i, []).append(("proj", i))

            def emit_S(u):
                g, h, p_ = units[u]
                j0 = 2 * p_
                w = _width(g, j0)
                hb = (h % 2) * 64
                pS = ps.tile([P, 1024], F32, tag="pS", bufs=2, name="pS")
                for jj in (0, 1):
                    j = j0 + jj
                    nc.tensor.matmul(
                        pS[:, jj * 512 : jj * 512 + w],
                        KT[hb : hb + 64, h // 2, j * P : (j + 1) * P],
                        QT[hb : hb + 64, h // 2, g * 512 : g * 512 + w],
                        start=True,
                        stop=True,
                    )
                return pS

            def emit_division_copy(pO):
                # stage 1, right after the group's last AV: denominators row
                # (accumulated via the Vx ones column) to SBUF. Runs on DVE
                # while the next unit's scores occupy the PE, so stage 2's
                # broadcast matmul doesn't head-of-line block the PE queue.
                # (A broadcast-DMA variant measured slower: the HWDGE+DMA
                # latency in the chain outweighs the saved DVE/PE ops.)
                rr = work.tile([P, 512], BF, tag="rr", bufs=2, name="rr")
                with nc.allow_low_precision(
                    reason="softmax denominators: bf16 is ample (~0.4% on a "
                    "per-query scale factor)"
                ):
                    nc.vector.tensor_copy(rr[64:65, :], pO[64:65, :])
                return rr

            def emit_division(h, g, pO, rr):
                hb = (h % 2) * 64
                # stage 2: broadcast across 64 partitions with a K=1 matmul
                # into the bank's unused upper rows (bf16: a f32 matmul costs
                # 4 cycles/row), reciprocal into SBUF, then one multiply
                # straight into z^T (a DVE op may read at most one PSUM
                # operand)
                with nc.allow_low_precision(
                    reason="softmax denominators: bf16 is ample (~0.4% on a "
                    "per-query scale factor)"
                ):
                    nc.tensor.matmul(
                        pO[64:128, :], ones64[64:65, :], rr[64:65, :],
                        start=True, stop=True,
                    )
                    nc.vector.reciprocal(rr[0:64, :], pO[64:128, :])
                nc.vector.tensor_tensor(
                    out=zT[hb : hb + 64, h // 2, g * 512 : (g + 1) * 512],
                    in0=pO[0:HD, :], in1=rr[0:64, :],
                    op=mybir.AluOpType.mult,
                )

            def emit_residual(g, chunks):
                cols = slice(g * 512, (g + 1) * 512)
                for c in chunks:
                    nc.vector.tensor_tensor(
                        out=zT[:, c, cols], in0=zT[:, c, cols],
                        in1=xTq[:, c, cols], op=mybir.AluOpType.add,
                    )

            inv_e = 1.0 / float(E)

            def emit_proj_tb(tb, last=False):
                # bias-add fused with row-sum accumulation (mean), variance
                # via Square(y - mu) with accum, final normalize as one
                # scale+bias activation; gamma/beta on the idle Pool engine
                # except for the last block (shortest critical chain on DVE)
                y_sb = work.tile([P, E], F32, tag="ysb", bufs=2, name="y_sb")
                s0 = work.tile([P, 1], F32, tag="stat", bufs=16, name="s0")
                ysum = work.tile([P, 1], F32, tag="stat", bufs=16, name="ysum")
                for fs in range(E // 512):
                    py = ps.tile([P, 512], F32, tag="py", bufs=2, name="py")
                    for c in range(EC):
                        nc.tensor.matmul(
                            py[:],
                            zT[:, c, tb * P : (tb + 1) * P],
                            Wp[fs][:, c, :],
                            start=(c == 0),
                            stop=(c == EC - 1),
                        )
                    nc.vector.scalar_tensor_tensor(
                        out=y_sb[:, fs * 512 : (fs + 1) * 512],
                        in0=py[:],
                        scalar=0.0,
                        in1=cBF[:, OFF_BP + fs * 512 : OFF_BP + (fs + 1) * 512],
                        op0=mybir.AluOpType.add,
                        op1=mybir.AluOpType.add,
                        accum_out=(s0 if fs == 0 else ysum)[:, 0:1],
                    )
                negmu = work.tile([P, 1], F32, tag="stat", bufs=16, name="negmu")
                nc.vector.tensor_tensor(
                    out=negmu[:], in0=s0[:], in1=ysum[:], op=mybir.AluOpType.add
                )
                nc.vector.tensor_scalar_mul(negmu[:], negmu[:], -inv_e)
                # ycb doubles as bf16 scratch for the variance pass's unused
                # main output, then holds the normalized result; gamma/beta
                # run in bf16 (4x DVE mode) and y ships as bf16. Variance and
                # normalize stay on ACT: putting them on DVE delays the
                # division multiplies behind them in DVE's in-order queue,
                # which stalls the PE's pO rotation.
                ycb = work.tile([P, E], BF, tag="ycb", bufs=2, name="ycb")
                var = work.tile([P, 1], F32, tag="stat", bufs=16, name="var")
                nc.scalar.activation(
                    ycb[:], y_sb[:], mybir.ActivationFunctionType.Square,
                    bias=negmu[:, 0:1], accum_out=var[:],
                )
                rstd = work.tile([P, 1], F32, tag="stat", bufs=16, name="rstd")
                nc.vector.tensor_scalar(
                    out=rstd[:], in0=var[:], scalar1=inv_e, scalar2=float(EPS),
                    op0=mybir.AluOpType.mult, op1=mybir.AluOpType.add,
                )
                nc.scalar.activation(
                    rstd[:], rstd[:], mybir.ActivationFunctionType.Sqrt
                )
                nc.vector.reciprocal(rstd[:], rstd[:])
                nmr = work.tile([P, 1], F32, tag="stat", bufs=16, name="nmr")
                nc.vector.tensor_tensor(
                    out=nmr[:], in0=negmu[:], in1=rstd[:], op=mybir.AluOpType.mult
                )
                nc.scalar.activation(
                    ycb[:], y_sb[:], mybir.ActivationFunctionType.Identity,
                    scale=rstd[:, 0:1], bias=nmr[:, 0:1],
                )
                nc.vector.tensor_tensor(
                    out=ycb[:], in0=ycb[:], in1=cBF[:, OFF_G : OFF_G + E],
                    op=mybir.AluOpType.mult,
                )
                nc.vector.tensor_tensor(
                    out=ycb[:], in0=ycb[:], in1=cBF[:, OFF_B : OFF_B + E],
                    op=mybir.AluOpType.add,
                )
                nc.sync.dma_start(y_d[tb, :, :], ycb[:])

            pO_cur = None
            pending_divs = []
            prev_S = emit_S(0)
            for u, (g, h, p_) in enumerate(units):
                j0 = 2 * p_
                w = _width(g, j0)
                maxL = GLS[g][0]
                if p_ == 0:
                    if pending_divs:
                        pending_divs.pop(0)()
                    pO_cur = ps.tile([P, 512], F32, tag="pO", bufs=2, name="pO")
                pO = pO_cur
                pS = prev_S
                eS = work.tile([P, 1024], BF, tag="eS", bufs=3, name="eS")
                nc.scalar.activation(
                    eS[:, :].rearrange("p (u q) -> p u q", u=2)[:, :, 0:w],
                    pS[:, :].rearrange("p (u q) -> p u q", u=2)[:, :, 0:w],
                    mybir.ActivationFunctionType.Exp,
                    scale=SCALE,
                )
                if u + 1 < len(units):
                    prev_S = emit_S(u + 1)
                if u == PART2_AT:
                    # all g0 divisions and heads 0-7's g1 divisions are done
                    emit_residual(0, range(EC))
                    emit_residual(1, range(4))
                for kind, arg in fillers.get(u, ()):
                    if kind == "v":
                        emit_v_group(arg, 1, "py")
                    elif kind == "v0":
                        emit_v_group(arg, 0, "py")
                    elif kind == "q":
                        emit_q_group(4 + arg // 2, arg % 2, "py")
                    else:
                        emit_proj_tb(arg)
                mi = MASK_IDX.get((g, j0))
                if mi is not None:
                    # both key blocks of the pair mask the same (last active)
                    # q-block, and their mask instances are consecutive in
                    # cBF: one 3D-AP multiply covers both halves
                    idx, bi = mi
                    assert MASK_IDX[(g, j0 + 1)] == (idx + 1, bi)
                    eSv = eS[:, :].rearrange("p (u q) -> p u q", u=2)[
                        :, :, bi * P : (bi + 1) * P
                    ]
                    nc.vector.tensor_tensor(
                        out=eSv, in0=eSv,
                        in1=cBF[:, OFF_M + idx * P : OFF_M + (idx + 2) * P]
                        .rearrange("p (u q) -> p u q", u=2),
                        op=mybir.AluOpType.mult,
                    )
                for jj in (0, 1):
                    j = j0 + jj
                    nc.tensor.matmul(
                        pO[0 : HD + 1, 0:w],
                        Vx[:, j, h, :],
                        eS[:, jj * 512 : jj * 512 + w],
                        start=(j == 0),
                        stop=(j == maxL - 1),
                        skip_group_check=True,
                    )
                if j0 + 1 == maxL - 1:
                    rr_cur = emit_division_copy(pO)

                    def _div(h=h, g=g, pO=pO, rr=rr_cur):
                        emit_division(h, g, pO, rr)
                        if g == 1 and h >= 9 and h % 2 == 1:
                            # z^T feature chunk h//2 complete for both column
                            # groups: add the residual now so the tail
                            # projection's contraction can start early
                            emit_residual(1, [h // 2])

                    pending_divs.append(_div)
            while pending_divs:
                pending_divs.pop(0)()

            # ---------------- tail: residual + projection for group 1 ------
            for tb in range(4, NQ):
                emit_proj_tb(tb, last=(tb == NQ - 1))

            _psB_cm.__exit__(None, None, None)

    _nc_cache["nc"] = nc
    return nc


def _make_mall(ownd):
    """Mask tiles for this core's descending-ordered q-blocks.

    Instance (g, j, bi): multiply eS columns of boundary q-block bi at key
    block j. Pattern depends on whether the block's true length equals the
    padded length (l_true == L) or falls one short (l_true == L-1)."""
    tril_t = (np.arange(P)[:, None] <= np.arange(P)[None, :]).astype(np.float32)
    mall = np.zeros((16, P, P), np.float32)
    for idx, (g, j, bi) in enumerate(MASK_INST):
        L = GLS[g][bi]
        block = ownd[g * 4 + bi]
        l_true = block + 1
        assert l_true in (L, L - 1)
        if j == L - 2:
            mall[idx] = 1.0 if l_true == L else tril_t
        else:
            mall[idx] = tril_t if l_true == L else 0.0
    # device layout [P(k-local), 16, P(q-local)]
    return np.ascontiguousarray(mall.transpose(1, 0, 2)).astype(NPBF)


def kernel(x, Wq, bq, Wk, bk, Wv, bv, Wp, bp, gamma, beta):
    x = np.asarray(x, np.float32)
    nc = _build_nc()

    WqT = np.ascontiguousarray(np.asarray(Wq, np.float32).T * WSCALE).astype(NPF8)
    WkT = np.ascontiguousarray(np.asarray(Wk, np.float32).T * WSCALE).astype(NPF8)
    WvT = np.ascontiguousarray(np.asarray(Wv, np.float32).T * WSCALE).astype(NPF8)
    WpT = np.ascontiguousarray(np.asarray(Wp, np.float32).T).astype(NPBF)
    bqT = np.ascontiguousarray(np.asarray(bq, np.float32).reshape(EC, P).T)
    bkT = np.ascontiguousarray(np.asarray(bk, np.float32).reshape(EC, P).T)
    cF32 = np.concatenate([bkT, bqT], axis=1)  # [P, 16]
    bcast4 = [
        np.broadcast_to(np.asarray(v, np.float32), (P, E))
        for v in (bv, bp, gamma, beta)
    ]
    # descending padded length = reversed block list
    ownd_map = {0: list(reversed(BLOCKS_A)), 1: list(reversed(BLOCKS_B))}
    cBF_map = {
        hh: np.ascontiguousarray(
            np.concatenate(
                bcast4 + [_make_mall(ownd_map[hh]).reshape(P, 16 * P)], axis=1
            )
        ).astype(NPBF)
        for hh in (0, 1)
    }

    in_maps = []
    for core in range(8):
        b, hh = core // 2, core % 2
        ownd = ownd_map[hh]
        own = np.concatenate([np.arange(blk * P, (blk + 1) * P) for blk in ownd])
        xb = x[b]  # (T, E)
        xT = np.ascontiguousarray(xb.T).astype(NPF8)
        # bv folded in: each head's output picks up exactly +bv after the
        # softmax division (weights sum to 1), so it lands here instead
        xTq = np.ascontiguousarray(
            xb[own].T + np.asarray(bv, np.float32)[:, None]
        ).astype(NPBF)
        xT8q = np.ascontiguousarray(xb[own].T).astype(NPF8)
        in_maps.append(
            {
                "xT": xT,
                "xTq": xTq,
                "xT8q": xT8q,
                "WqT": WqT,
                "WkT": WkT,
                "WvT": WvT,
                "WpT": WpT,
                "cF32": cF32,
                "cBF": cBF_map[hh],
            }
        )

    import os

    trace = bool(int(os.environ.get("MHSA_TRACE", "0")))
    res = run_bass_kernel_spmd(
        nc, in_maps, core_ids=list(range(8)), trace=trace,
        trace_cores=list(range(8)) if trace else None,
    )
    if trace and res.exec_time_ns is not None:
        print(f"HW exec time: {res.exec_time_ns} ns")
        if res.mean_exec_time_ns is not None:
            print(f"HW exec mean across cores: {res.mean_exec_time_ns:.0f} ns")
        kernel.last_exec_time_ns = res.exec_time_ns
        kernel.last_trace = res.instructions_and_trace

    out = np.empty((B, T, E), np.float32)
    for core in range(8):
        b, hh = core // 2, core % 2
        ownd = ownd_map[hh]
        y = res.results[core]["y"]  # (NQ, P, E) bf16
        for k, blk in enumerate(ownd):
            out[b, blk * P : (blk + 1) * P, :] = y[k].astype(np.float32)
    return out


# revision 109
# speedup vs baseline: 1.0040x; 1.0040x over previous
"""Multi-head self-attention (B=4, T=2048, E=1024, H=16) on 8 trn2 NeuronCores.

Sharding: core (b, h) = batch b, token-half h. Each core computes K/V for the
full sequence (duplicated within the batch pair), Q for its own 8 query blocks
of 128 tokens, causal attention for those blocks, then the output projection
and LayerNorm for its own tokens.

Attention restructure (vs the 128-wide-per-head-pair baseline): each core's
query blocks are ordered by DESCENDING padded causal length (16,14,12,10 |
8,6,4,2 key blocks), so for key block j the active query blocks form a
contiguous prefix. Scores/AV run one matmul per (head, group-of-4-q-blocks,
key block) with free dim up to 512, cutting PE instruction count ~3x. The
softmax denominator division runs once per (head, group) on 512 columns.
Projection+LN for the first 4 token blocks is interleaved into the second
(light) attention group to shrink the tail.

Causal balance: query blocks are paired (j, 15-j) so both cores of a batch
process blocks with padded key-lengths 2,4,...,16; host-supplied mask tiles
encode the true causal structure, keeping the compiled program identical
across cores (SPMD).

All matmuls run in bf16 with fp32 PSUM accumulation (validated ~2e-3
scale-relative error vs the fp32 reference).
"""
import json
import numpy as np
import ml_dtypes
from contextlib import ExitStack

import concourse.bass as bass
import concourse.bass_utils as _bass_utils
import concourse.tile as tile
from concourse import mybir
from concourse.bass_utils import run_bass_kernel_spmd

# ----------------------------------------------------------------------------
# Toolchain workarounds for this container's walrus build (see birfix notes):
# 1. EVENT_SEMAPHORE_RANGE_CLEAR InstISA is rejected ("ISA wrong length").
# 2. Engine instructions only carry one semaphore-wait slot; extra waits are
#    peeled onto NoOp carriers on the same engine (order-preserving).
# ----------------------------------------------------------------------------


def _patched_clear_and_free_semaphores(self, sems):
    if not sems:
        return
    sem_nums = [s.num if hasattr(s, "num") else s for s in sems]
    self._state.prepend_free_semaphores(sem_nums)
    for poison_set in self._tile_sem_poison_stack:
        poison_set.update(sem_nums)


def _fix_bir_waits(bir_json: bytes) -> bytes:
    bir = json.loads(bir_json)
    ctr = 0
    changed = False
    for func in bir.get("functions", []):
        for blk in func.get("blocks", []):
            out = []
            for inst in blk.get("instructions", []):
                si = inst.get("sync_info") or {}
                waits = si.get("on_wait") or []
                if len(waits) > 1:
                    for w in waits[:-1]:
                        ctr += 1
                        out.append(
                            {
                                "debug": inst.get("debug"),
                                "engine": inst.get("engine", "SP"),
                                "ins": [],
                                "name": f"IWF-{ctr}",
                                "opcode": "NoOp",
                                "outs": [],
                                "sync_info": {"on_wait": [w]},
                            }
                        )
                    si = dict(si)
                    si["on_wait"] = waits[-1:]
                    inst = dict(inst)
                    inst["sync_info"] = si
                    changed = True
                out.append(inst)
            blk["instructions"] = out
    return json.dumps(bir).encode() if changed else bir_json


_orig_compile_bir_kernel = _bass_utils.compile_bir_kernel


def _patched_compile_bir_kernel(bir_json, tmpdir, neff_name="file.neff"):
    if isinstance(bir_json, str):
        bir_json = bir_json.encode()
    return _orig_compile_bir_kernel(_fix_bir_waits(bir_json), tmpdir, neff_name)


def _install_patches():
    if getattr(bass.Bass, "_mhsa_patched", False):
        return
    bass.Bass.clear_and_free_semaphores = _patched_clear_and_free_semaphores
    bass.Bass._mhsa_patched = True
    _bass_utils.compile_bir_kernel = _patched_compile_bir_kernel
    try:
        import concourse.bass2jax as _b2j

        _b2j.compile_bir_kernel = _patched_compile_bir_kernel
    except ImportError:
        pass


_install_patches()

# ----------------------------------------------------------------------------
# Problem constants (hardcoded per spec)
# ----------------------------------------------------------------------------
B, T, E, H = 4, 2048, 1024, 16
HD = E // H  # 64
P = 128
NB = T // P  # 16 query/key blocks
NQ = 8  # query blocks per core
EC = E // P  # 8 e-chunks
SCALE = 1.0 / float(np.sqrt(T))
EPS = 1e-6
BF = mybir.dt.bfloat16
F32 = mybir.dt.float32
F8 = mybir.dt.float8e4
NPBF = ml_dtypes.bfloat16
NPF8 = ml_dtypes.float8_e4m3
# fp8 weights are pre-scaled by 16 host-side (keeps N(0, 0.02) entries out
# of the fp8e4m3 subnormal range); the PSUM drain multiplies by 1/16
WSCALE = 16.0
WINV = 1.0 / WSCALE
DR = mybir.MatmulPerfMode.DoubleRow

# query-block assignment: pairs (j, 15-j) so both cores of a batch pair see
# padded lengths {2,4,...,16}; blocks listed in ASCENDING padded length
BLOCKS_A = [0, 2, 4, 6, 9, 11, 13, 15]  # true lengths 1,3,5,7,10,12,14,16
BLOCKS_B = [1, 3, 5, 7, 8, 10, 12, 14]  # true lengths 2,4,6,8,9,11,13,15

# device-side q-block order: DESCENDING padded length; two groups of 4
GLS = {0: (16, 14, 12, 10), 1: (8, 6, 4, 2)}
# mask instances: (group, key block j, boundary q-block index bi); the
# boundary block is always the LAST active block of the prefix at that j
MASK_INST = []
for _g in (0, 1):
    for _j in range(GLS[_g][0]):
        for _bi, _L in enumerate(GLS[_g]):
            if _j in (_L - 2, _L - 1):
                MASK_INST.append((_g, _j, _bi))
MASK_IDX = {(g, j): (idx, bi) for idx, (g, j, bi) in enumerate(MASK_INST)}
assert len(MASK_INST) == 16


def _width(g, j):
    return 128 * sum(1 for L in GLS[g] if L > j)


_nc_cache = {}


def _build_nc():
    if "nc" in _nc_cache:
        return _nc_cache["nc"]
    nc = bass.Bass(num_devices=8)

    # inputs (per-core)
    xT_d = nc.dram_tensor("xT", [E, T], F8, kind="ExternalInput")
    xTq_d = nc.dram_tensor("xTq", [E, NQ * P], BF, kind="ExternalInput")
    xT8q_d = nc.dram_tensor("xT8q", [E, NQ * P], F8, kind="ExternalInput")
    WqT_d = nc.dram_tensor("WqT", [E, E], F8, kind="ExternalInput")
    WkT_d = nc.dram_tensor("WkT", [E, E], F8, kind="ExternalInput")
    WvT_d = nc.dram_tensor("WvT", [E, E], F8, kind="ExternalInput")
    WpT_d = nc.dram_tensor("WpT", [E, E], BF, kind="ExternalInput")
    cF32_d = nc.dram_tensor("cF32", [P, 16], F32, kind="ExternalInput")
    cBF_d = nc.dram_tensor("cBF", [P, 4 * E + 16 * P], BF, kind="ExternalInput")
    y_d = nc.dram_tensor("y", [NQ, P, E], BF, kind="ExternalOutput")

    with tile.TileContext(nc) as tc:
        with ExitStack() as ctx:
            consts = ctx.enter_context(tc.tile_pool(name="consts", bufs=1))
            big = ctx.enter_context(tc.tile_pool(name="big", bufs=1))
            wpool = ctx.enter_context(tc.tile_pool(name="wpool", bufs=1))
            # xT is only needed during the QKV phase; its pool is closed
            # before the attention working set is allocated
            xtp = ctx.enter_context(tc.tile_pool(name="xtp", bufs=1))
            _psA_cm = tc.tile_pool(name="psA", bufs=1, space="PSUM")
            ps = _psA_cm.__enter__()

            def load_w(dram, name, interleave_with=None, split_first=False,
                       dtype=F8):
                # two half-tiles in a 3-slot rotation: the next projection's
                # first half streams in while the previous one's second half
                # is still being consumed. ONE DMA per half (HWDGE issue is a
                # serialized ~625ns/DMA shared resource — minimize count)
                halves = []
                for hf in range(2):
                    w = wpool.tile(
                        [P, EC, E // 2], dtype, tag="wh", bufs=3,
                        name=f"{name}{hf}"
                    )
                    if split_first and hf == 0:
                        # first fb-chunk separately: the first K matmul group
                        # only needs cols 0:128, so it starts ~3us earlier
                        nc.sync.dma_start(
                            w[:, :, 0:P],
                            dram.rearrange("(c p) f -> p c f", p=P)[:, :, 0:P],
                        )
                        if interleave_with is not None:
                            interleave_with(hf)
                        nc.sync.dma_start(
                            w[:, :, P:512],
                            dram.rearrange("(c p) f -> p c f", p=P)[:, :, P:512],
                        )
                    else:
                        nc.sync.dma_start(
                            w[:, :, :],
                            dram.rearrange("(c p) f -> p c f", p=P)[
                                :, :, hf * 512 : (hf + 1) * 512
                            ],
                        )
                        if interleave_with is not None:
                            interleave_with(hf)
                    halves.append(w)
                return halves

            # PE-critical loads first. HWDGE queue order: Wk half0, xT win0,
            # f32 consts (bk needed by the first bias add), Wk half1, then
            # the remaining xT windows — the ts-outer K loop consumes one
            # window per ~13.7us so the serialized DMA stream stays ahead
            xT = xtp.tile([P, EC, T], F8)
            cF32 = consts.tile([P, 16], F32)
            cBF = consts.tile([P, 4 * E + 16 * P], BF)

            def _xt_w(wi):
                nc.sync.dma_start(
                    xT[:, :, wi * 512 : (wi + 1) * 512],
                    xT_d.rearrange("(c p) t -> p c t", p=P)[
                        :, :, wi * 512 : (wi + 1) * 512
                    ],
                )

            def _wk_companion(hf):
                if hf == 0:
                    _xt_w(0)
                    nc.sync.dma_start(cF32[:, :], cF32_d[:, :])

            # serialized-DMA ordering: everything attention-start needs goes
            # first (xT8q for Q, cBF for masks, Wv); the big bf16 xTq is only
            # read by the residual ~150us in, so its DMA is issued after all
            # weight loads (tile allocated here, transfer deferred)
            Wk = load_w(WkT_d, "Wk", interleave_with=_wk_companion, split_first=True)
            _xt_w(1)
            _xt_w(2)
            _xt_w(3)
            xT8q = big.tile([P, EC, NQ * P], F8)
            nc.sync.dma_start(
                xT8q[:, :, :], xT8q_d.rearrange("(c p) t -> p c t", p=P)[:, :, :]
            )
            nc.sync.dma_start(cBF[:, :], cBF_d[:, :])
            xTq = big.tile([P, EC, NQ * P], BF)
            wvp = ctx.enter_context(tc.tile_pool(name="wvp", bufs=1))
            Wv = []
            for hf in range(2):
                wv = wvp.tile([P, EC, E // 2], F8, tag=f"wv{hf}", bufs=1,
                              name=f"Wv{hf}")
                nc.sync.dma_start(
                    wv[:, :, :],
                    WvT_d.rearrange("(c p) f -> p c f", p=P)[
                        :, :, hf * 512 : (hf + 1) * 512
                    ],
                )
                Wv.append(wv)
            # packed-constant layout in cBF: bv | bp | gamma | beta | masks
            OFF_BV, OFF_BP, OFF_G, OFF_B, OFF_M = 0, E, 2 * E, 3 * E, 4 * E

            def mall_at(idx):
                return cBF[:, OFF_M + idx * P : OFF_M + (idx + 1) * P]

            ones64 = consts.tile([P, 64], BF)
            nc.vector.memset(ones64[:], 1.0)

            # persistent intermediates
            KT = big.tile([P, EC, T], BF)  # K^T  [f, t]
            QT = big.tile([P, EC, NQ * P], BF)  # Q^T  [f, t_own]
            Vx = big.tile([P, NB, H, HD + 1], BF)  # V ext [t, h, d|1]
            zT = big.tile([P, EC, NQ * P], BF)  # z^T  [e, t_own]
            nc.vector.memset(Vx[:, :, :, HD : HD + 1], 1.0)

            # ---------------- K^T = Wk^T.T-chunks x xT + bk ----------------
            # ts outer: each xT window feeds all 8 fb groups (~13.7us of PE
            # work) so the next window's DMA completes in the shadow
            for ts_ in range(T // 512):
                for fb in range(EC):
                    pk = ps.tile([P, 512], F32, tag="mm512", bufs=6, name="pk")
                    for c in range(0, EC, 2):
                        nc.tensor.matmul(
                            pk[:],
                            Wk[fb // 4][:, c : c + 2, (fb % 4) * P : (fb % 4 + 1) * P],
                            xT[:, c : c + 2, ts_ * 512 : (ts_ + 1) * 512],
                            start=(c == 0),
                            stop=(c == EC - 2),
                            perf_mode=DR,
                        )
                    # drains alternate DVE/ACT (ACT is idle until attention;
                    # K^T partitions are features, so the bias is
                    # per-partition — exactly activation's bias operand).
                    # With fp8 the PE is 4x faster here and a single drain
                    # engine becomes the QKV-phase bottleneck.
                    if fb % 2 == 0:
                        nc.scalar.activation(
                            KT[:, fb, ts_ * 512 : (ts_ + 1) * 512],
                            pk[:],
                            mybir.ActivationFunctionType.Identity,
                            scale=WINV,
                            bias=cF32[:, fb : fb + 1],
                        )
                    else:
                        nc.vector.tensor_scalar(
                            out=KT[:, fb, ts_ * 512 : (ts_ + 1) * 512],
                            in0=pk[:],
                            scalar1=WINV,
                            scalar2=cF32[:, fb : fb + 1],
                            op0=mybir.AluOpType.mult,
                            op1=mybir.AluOpType.add,
                        )

            # ---------------- V (heads 0-7, blocks 0-7) ---------------------
            # V blocks 8-15, V heads 8-15, and Q feature-blocks 4-7 are
            # deferred: their matmul groups interleave into the exp-bound
            # attention wave, keeping PE busy while the Activation engine
            # catches up on exponentials. Wv lives in its own persistent pool
            # because its readers now extend deep into the attention phase
            # (the 3-slot weight rotation would reuse its slots too early).
            def emit_v_group(tb, fs, pool_tag):
                pv = ps.tile([P, 512], F32, tag=pool_tag, bufs=6 if pool_tag == "mm512" else 2, name="pv")
                for c in range(0, EC, 2):
                    nc.tensor.matmul(
                        pv[:],
                        xT[:, c : c + 2, tb * P : (tb + 1) * P],
                        Wv[fs][:, c : c + 2, :],
                        start=(c == 0),
                        stop=(c == EC - 2),
                        perf_mode=DR,
                    )
                # bv is NOT added here: softmax rows sum to 1, so the bias
                # contributes exactly bv per feature after normalization —
                # the host pre-adds it into the residual input xTq instead.
                # Scale-only drain; pre-phase drains alternate onto the
                # still-idle ACT engine.
                if pool_tag == "mm512" and tb % 2 == 0:
                    nc.scalar.activation(
                        Vx[:, tb, fs * 8 : (fs + 1) * 8, 0:HD],
                        pv[:, :].rearrange("p (h d) -> p h d", d=HD),
                        mybir.ActivationFunctionType.Identity,
                        scale=WINV,
                    )
                else:
                    nc.vector.tensor_scalar_mul(
                        Vx[:, tb, fs * 8 : (fs + 1) * 8, 0:HD],
                        pv[:, :].rearrange("p (h d) -> p h d", d=HD),
                        WINV,
                    )

            for tb in range(8):
                emit_v_group(tb, 0, "mm512")

            # ---------------- Q^T (fb 0-3) = Wq^T-chunks x xTq + bq --------
            Wq = load_w(WqT_d, "Wq")

            def emit_q_group(fb, ts_, pool_tag):
                pq = ps.tile([P, 512], F32, tag=pool_tag, bufs=6 if pool_tag == "mm512" else 2, name="pq")
                for c in range(0, EC, 2):
                    nc.tensor.matmul(
                        pq[:],
                        Wq[fb // 4][:, c : c + 2, (fb % 4) * P : (fb % 4 + 1) * P],
                        xT8q[:, c : c + 2, ts_ * 512 : (ts_ + 1) * 512],
                        start=(c == 0),
                        stop=(c == EC - 2),
                        perf_mode=DR,
                    )
                if pool_tag == "mm512" and fb % 2 == 0:
                    # pre-phase only: ACT is idle before attention starts
                    nc.scalar.activation(
                        QT[:, fb, ts_ * 512 : (ts_ + 1) * 512],
                        pq[:],
                        mybir.ActivationFunctionType.Identity,
                        scale=WINV,
                        bias=cF32[:, EC + fb : EC + fb + 1],
                    )
                else:
                    nc.vector.tensor_scalar(
                        out=QT[:, fb, ts_ * 512 : (ts_ + 1) * 512],
                        in0=pq[:],
                        scalar1=WINV,
                        scalar2=cF32[:, EC + fb : EC + fb + 1],
                        op0=mybir.AluOpType.mult,
                        op1=mybir.AluOpType.add,
                    )

            for fb in range(4):
                for ts_ in range(NQ * P // 512):
                    emit_q_group(fb, ts_, "mm512")

            # Wp streams in during attention (3-slot rotation frees Wq slots)
            Wp = load_w(WpT_d, "Wp", dtype=BF)
            # deferred residual input (first read at wave B part 2)
            nc.sync.dma_start(
                xTq[:, :, :], xTq_d.rearrange("(c p) t -> p c t", p=P)[:, :, :]
            )

            # ---------------- attention ----------------
            work = ctx.enter_context(tc.tile_pool(name="work", bufs=2))
            _psA_cm.__exit__(None, None, None)
            _psB_cm = tc.tile_pool(name="psB", bufs=1, space="PSUM")
            ps = _psB_cm.__enter__()

            # unit schedule: wave A = heads 0-7 (g0 then g1), wave B = heads
            # 8-15 g0, residual(0), heads 8-15 g1 with proj tb0-3 interleaved
            units = []
            for h in range(8):
                for g in (0, 1):
                    for p_ in range(GLS[g][0] // 2):
                        units.append((g, h, p_))
            for h in range(8, H):
                for p_ in range(GLS[0][0] // 2):
                    units.append((0, h, p_))
            for h in range(8, H):
                for p_ in range(GLS[1][0] // 2):
                    units.append((1, h, p_))
            WAVE_A_N = 96  # units in wave A
            PART2_AT = WAVE_A_N + 64  # first (g1, h>=8) unit: residual(0) here

            # fillers: V key-blocks 8-15 for heads 0-7 land up front (needed
            # by h0's AV from unit 4 on), then deferred V heads 8-15 and
            # Q fb 4-7 spread through wave A and slightly into wave B part 1;
            # projection tb0-3 spread through part 2
            fillers = {}
            for tb in range(8, NB):
                fillers.setdefault((tb - 8) // 2, []).append(("v0", tb))
            deferred = []
            for tb in range(NB):
                deferred.append(("v", tb))
                if tb < 8:
                    deferred.append(("q", tb))
            for i, d in enumerate(deferred):
                fillers.setdefault(
                    4 + i * (WAVE_A_N + 1) // len(deferred), []
                ).append(d)
            for i in range(4):
                fillers.setdefault(PART2_AT + 6 + 8 * i, []).append(("proj", i))

            def emit_S(u):
                g, h, p_ = units[u]
                j0 = 2 * p_
                w = _width(g, j0)
                hb = (h % 2) * 64
                pS = ps.tile([P, 1024], F32, tag="pS", bufs=2, name="pS")
                for jj in (0, 1):
                    j = j0 + jj
                    nc.tensor.matmul(
                        pS[:, jj * 512 : jj * 512 + w],
                        KT[hb : hb + 64, h // 2, j * P : (j + 1) * P],
                        QT[hb : hb + 64, h // 2, g * 512 : g * 512 + w],
                        start=True,
                        stop=True,
                    )
                return pS

            def emit_division_copy(pO):
                # stage 1, right after the group's last AV: denominators row
                # (accumulated via the Vx ones column) to SBUF. Runs on DVE
                # while the next unit's scores occupy the PE, so stage 2's
                # broadcast matmul doesn't head-of-line block the PE queue.
                # (A broadcast-DMA variant measured slower: the HWDGE+DMA
                # latency in the chain outweighs the saved DVE/PE ops.)
                rr = work.tile([P, 512], BF, tag="rr", bufs=2, name="rr")
                with nc.allow_low_precision(
                    reason="softmax denominators: bf16 is ample (~0.4% on a "
                    "per-query scale factor)"
                ):
                    nc.vector.tensor_copy(rr[64:65, :], pO[64:65, :])
                return rr

            def emit_division(h, g, pO, rr):
                hb = (h % 2) * 64
                # stage 2: broadcast across 64 partitions with a K=1 matmul
                # into the bank's unused upper rows (bf16: a f32 matmul costs
                # 4 cycles/row), reciprocal into SBUF, then one multiply
                # straight into z^T (a DVE op may read at most one PSUM
                # operand)
                with nc.allow_low_precision(
                    reason="softmax denominators: bf16 is ample (~0.4% on a "
                    "per-query scale factor)"
                ):
                    nc.tensor.matmul(
                        pO[64:128, :], ones64[64:65, :], rr[64:65, :],
                        start=True, stop=True,
                    )
                    nc.vector.reciprocal(rr[0:64, :], pO[64:128, :])
                nc.vector.tensor_tensor(
                    out=zT[hb : hb + 64, h // 2, g * 512 : (g + 1) * 512],
                    in0=pO[0:HD, :], in1=rr[0:64, :],
                    op=mybir.AluOpType.mult,
                )

            def emit_residual(g, chunks):
                cols = slice(g * 512, (g + 1) * 512)
                for c in chunks:
                    nc.vector.tensor_tensor(
                        out=zT[:, c, cols], in0=zT[:, c, cols],
                        in1=xTq[:, c, cols], op=mybir.AluOpType.add,
                    )

            inv_e = 1.0 / float(E)

            def emit_proj_tb(tb, last=False):
                # bias-add fused with row-sum accumulation (mean), variance
                # via Square(y - mu) with accum, final normalize as one
                # scale+bias activation; gamma/beta on the idle Pool engine
                # except for the last block (shortest critical chain on DVE)
                y_sb = work.tile([P, E], F32, tag="ysb", bufs=2, name="y_sb")
                s0 = work.tile([P, 1], F32, tag="stat", bufs=16, name="s0")
                ysum = work.tile([P, 1], F32, tag="stat", bufs=16, name="ysum")
                for fs in range(E // 512):
                    py = ps.tile([P, 512], F32, tag="py", bufs=2, name="py")
                    for c in range(EC):
                        nc.tensor.matmul(
                            py[:],
                            zT[:, c, tb * P : (tb + 1) * P],
                            Wp[fs][:, c, :],
                            start=(c == 0),
                            stop=(c == EC - 1),
                        )
                    nc.vector.scalar_tensor_tensor(
                        out=y_sb[:, fs * 512 : (fs + 1) * 512],
                        in0=py[:],
                        scalar=0.0,
                        in1=cBF[:, OFF_BP + fs * 512 : OFF_BP + (fs + 1) * 512],
                        op0=mybir.AluOpType.add,
                        op1=mybir.AluOpType.add,
                        accum_out=(s0 if fs == 0 else ysum)[:, 0:1],
                    )
                negmu = work.tile([P, 1], F32, tag="stat", bufs=16, name="negmu")
                nc.vector.tensor_tensor(
                    out=negmu[:], in0=s0[:], in1=ysum[:], op=mybir.AluOpType.add
                )
                nc.vector.tensor_scalar_mul(negmu[:], negmu[:], -inv_e)
                # ycb doubles as bf16 scratch for the variance pass's unused
                # main output, then holds the normalized result; gamma/beta
                # run in bf16 (4x DVE mode) and y ships as bf16. Variance and
                # normalize stay on ACT: putting them on DVE delays the
                # division multiplies behind them in DVE's in-order queue,
                # which stalls the PE's pO rotation.
                ycb = work.tile([P, E], BF, tag="ycb", bufs=2, name="ycb")
                var = work.tile([P, 1], F32, tag="stat", bufs=16, name="var")
                nc.scalar.activation(
                    ycb[:], y_sb[:], mybir.ActivationFunctionType.Square,
                    bias=negmu[:, 0:1], accum_out=var[:],
                )
                rstd = work.tile([P, 1], F32, tag="stat", bufs=16, name="rstd")
                nc.vector.tensor_scalar(
                    out=rstd[:], in0=var[:], scalar1=inv_e, scalar2=float(EPS),
                    op0=mybir.AluOpType.mult, op1=mybir.AluOpType.add,
                )
                nc.scalar.activation(
                    rstd[:], rstd[:], mybir.ActivationFunctionType.Sqrt
                )
                nc.vector.reciprocal(rstd[:], rstd[:])
                nmr = work.tile([P, 1], F32, tag="stat", bufs=16, name="nmr")
                nc.vector.tensor_tensor(
                    out=nmr[:], in0=negmu[:], in1=rstd[:], op=mybir.AluOpType.mult
                )
                nc.scalar.activation(
                    ycb[:], y_sb[:], mybir.ActivationFunctionType.Identity,
                    scale=rstd[:, 0:1], bias=nmr[:, 0:1],
                )
                nc.vector.tensor_tensor(
                    out=ycb[:], in0=ycb[:], in1=cBF[:, OFF_G : OFF_G + E],
                    op=mybir.AluOpType.mult,
                )
                nc.vector.tensor_tensor(
                    out=ycb[:], in0=ycb[:], in1=cBF[:, OFF_B : OFF_B + E],
                    op=mybir.AluOpType.add,
                )
                nc.sync.dma_start(y_d[tb, :, :], ycb[:])

            pO_cur = None
            pending_divs = []
            prev_S = emit_S(0)
            for u, (g, h, p_) in enumerate(units):
                j0 = 2 * p_
                w = _width(g, j0)
                maxL = GLS[g][0]
                if p_ == 0:
                    if pending_divs:
                        pending_divs.pop(0)()
                    pO_cur = ps.tile([P, 512], F32, tag="pO", bufs=2, name="pO")
                pO = pO_cur
                pS = prev_S
                eS = work.tile([P, 1024], BF, tag="eS", bufs=3, name="eS")
                nc.scalar.activation(
                    eS[:, :].rearrange("p (u q) -> p u q", u=2)[:, :, 0:w],
                    pS[:, :].rearrange("p (u q) -> p u q", u=2)[:, :, 0:w],
                    mybir.ActivationFunctionType.Exp,
                    scale=SCALE,
                )
                if u + 1 < len(units):
                    prev_S = emit_S(u + 1)
                if u == PART2_AT:
                    # all g0 divisions and heads 0-7's g1 divisions are done
                    emit_residual(0, range(EC))
                    emit_residual(1, range(4))
                for kind, arg in fillers.get(u, ()):
                    if kind == "v":
                        emit_v_group(arg, 1, "py")
                    elif kind == "v0":
                        emit_v_group(arg, 0, "py")
                    elif kind == "q":
                        emit_q_group(4 + arg // 2, arg % 2, "py")
                    else:
                        emit_proj_tb(arg)
                mi = MASK_IDX.get((g, j0))
                if mi is not None:
                    # both key blocks of the pair mask the same (last active)
                    # q-block, and their mask instances are consecutive in
                    # cBF: one 3D-AP multiply covers both halves
                    idx, bi = mi
                    assert MASK_IDX[(g, j0 + 1)] == (idx + 1, bi)
                    eSv = eS[:, :].rearrange("p (u q) -> p u q", u=2)[
                        :, :, bi * P : (bi + 1) * P
                    ]
                    nc.vector.tensor_tensor(
                        out=eSv, in0=eSv,
                        in1=cBF[:, OFF_M + idx * P : OFF_M + (idx + 2) * P]
                        .rearrange("p (u q) -> p u q", u=2),
                        op=mybir.AluOpType.mult,
                    )
                for jj in (0, 1):
                    j = j0 + jj
                    nc.tensor.matmul(
                        pO[0 : HD + 1, 0:w],
                        Vx[:, j, h, :],
                        eS[:, jj * 512 : jj * 512 + w],
                        start=(j == 0),
                        stop=(j == maxL - 1),
                        skip_group_check=True,
                    )
                if j0 + 1 == maxL - 1:
                    rr_cur = emit_division_copy(pO)

                    def _div(h=h, g=g, pO=pO, rr=rr_cur):
                        emit_division(h, g, pO, rr)
                        if g == 1 and h >= 9 and h % 2 == 1:
                            # z^T feature chunk h//2 complete for both column
                            # groups: add the residual now so the tail
                            # projection's contraction can start early
                            emit_residual(1, [h // 2])

                    pending_divs.append(_div)
            while pending_divs:
                pending_divs.pop(0)()

            # ---------------- tail: residual + projection for group 1 ------
            for tb in range(4, NQ):
                emit_proj_tb(tb, last=(tb == NQ - 1))

            _psB_cm.__exit__(None, None, None)

    _nc_cache["nc"] = nc
    return nc


def _make_mall(ownd):
    """Mask tiles for this core's descending-ordered q-blocks.

    Instance (g, j, bi): multiply eS columns of boundary q-block bi at key
    block j. Pattern depends on whether the block's true length equals the
    padded length (l_true == L) or falls one short (l_true == L-1)."""
    tril_t = (np.arange(P)[:, None] <= np.arange(P)[None, :]).astype(np.float32)
    mall = np.zeros((16, P, P), np.float32)
    for idx, (g, j, bi) in enumerate(MASK_INST):
        L = GLS[g][bi]
        block = ownd[g * 4 + bi]
        l_true = block + 1
        assert l_true in (L, L - 1)
        if j == L - 2:
            mall[idx] = 1.0 if l_true == L else tril_t
        else:
            mall[idx] = tril_t if l_true == L else 0.0
    # device layout [P(k-local), 16, P(q-local)]
    return np.ascontiguousarray(mall.transpose(1, 0, 2)).astype(NPBF)


def kernel(x, Wq, bq, Wk, bk, Wv, bv, Wp, bp, gamma, beta):
    x = np.asarray(x, np.float32)
    nc = _build_nc()

    WqT = np.ascontiguousarray(np.asarray(Wq, np.float32).T * WSCALE).astype(NPF8)
    WkT = np.ascontiguousarray(np.asarray(Wk, np.float32).T * WSCALE).astype(NPF8)
    WvT = np.ascontiguousarray(np.asarray(Wv, np.float32).T * WSCALE).astype(NPF8)
    WpT = np.ascontiguousarray(np.asarray(Wp, np.float32).T).astype(NPBF)
    bqT = np.ascontiguousarray(np.asarray(bq, np.float32).reshape(EC, P).T)
    bkT = np.ascontiguousarray(np.asarray(bk, np.float32).reshape(EC, P).T)
    cF32 = np.concatenate([bkT, bqT], axis=1)  # [P, 16]
    bcast4 = [
        np.broadcast_to(np.asarray(v, np.float32), (P, E))
        for v in (bv, bp, gamma, beta)
    ]
    # descending padded length = reversed block list
    ownd_map = {0: list(reversed(BLOCKS_A)), 1: list(reversed(BLOCKS_B))}
    cBF_map = {
        hh: np.ascontiguousarray(
            np.concatenate(
                bcast4 + [_make_mall(ownd_map[hh]).reshape(P, 16 * P)], axis=1
            )
        ).astype(NPBF)
        for hh in (0, 1)
    }

    in_maps = []
    for core in range(8):
        b, hh = core // 2, core % 2
        ownd = ownd_map[hh]
        own = np.concatenate([np.arange(blk * P, (blk + 1) * P) for blk in ownd])
        xb = x[b]  # (T, E)
        xT = np.ascontiguousarray(xb.T).astype(NPF8)
        # bv folded in: each head's output picks up exactly +bv after the
        # softmax division (weights sum to 1), so it lands here instead
        xTq = np.ascontiguousarray(
            xb[own].T + np.asarray(bv, np.float32)[:, None]
        ).astype(NPBF)
        xT8q = np.ascontiguousarray(xb[own].T).astype(NPF8)
        in_maps.append(
            {
                "xT": xT,
                "xTq": xTq,
                "xT8q": xT8q,
                "WqT": WqT,
                "WkT": WkT,
                "WvT": WvT,
                "WpT": WpT,
                "cF32": cF32,
                "cBF": cBF_map[hh],
            }
        )

    import os

    trace = bool(int(os.environ.get("MHSA_TRACE", "0")))
    res = run_bass_kernel_spmd(
        nc, in_maps, core_ids=list(range(8)), trace=trace,
        trace_cores=list(range(8)) if trace else None,
    )
    if trace and res.exec_time_ns is not None:
        print(f"HW exec time: {res.exec_time_ns} ns")
        if res.mean_exec_time_ns is not None:
            print(f"HW exec mean across cores: {res.mean_exec_time_ns:.0f} ns")
        kernel.last_exec_time_ns = res.exec_time_ns
        kernel.last_trace = res.instructions_and_trace

    out = np.empty((B, T, E), np.float32)
    for core in range(8):
        b, hh = core // 2, core % 2
        ownd = ownd_map[hh]
        y = res.results[core]["y"]  # (NQ, P, E) bf16
        for k, blk in enumerate(ownd):
            out[b, blk * P : (blk + 1) * P, :] = y[k].astype(np.float32)
    return out


# revision 110
# speedup vs baseline: 1.0307x; 1.0266x over previous
"""Multi-head self-attention (B=4, T=2048, E=1024, H=16) on 8 trn2 NeuronCores.

Sharding: core (b, h) = batch b, token-half h. Each core computes K/V for the
full sequence (duplicated within the batch pair), Q for its own 8 query blocks
of 128 tokens, causal attention for those blocks, then the output projection
and LayerNorm for its own tokens.

Attention restructure (vs the 128-wide-per-head-pair baseline): each core's
query blocks are ordered by DESCENDING padded causal length (16,14,12,10 |
8,6,4,2 key blocks), so for key block j the active query blocks form a
contiguous prefix. Scores/AV run one matmul per (head, group-of-4-q-blocks,
key block) with free dim up to 512, cutting PE instruction count ~3x. The
softmax denominator division runs once per (head, group) on 512 columns.
Projection+LN for the first 4 token blocks is interleaved into the second
(light) attention group to shrink the tail.

Causal balance: query blocks are paired (j, 15-j) so both cores of a batch
process blocks with padded key-lengths 2,4,...,16; host-supplied mask tiles
encode the true causal structure, keeping the compiled program identical
across cores (SPMD).

All matmuls run in bf16 with fp32 PSUM accumulation (validated ~2e-3
scale-relative error vs the fp32 reference).
"""
import json
import numpy as np
import ml_dtypes
from contextlib import ExitStack

import concourse.bass as bass
import concourse.bass_utils as _bass_utils
import concourse.tile as tile
from concourse import mybir
from concourse.bass_utils import run_bass_kernel_spmd

# ----------------------------------------------------------------------------
# Toolchain workarounds for this container's walrus build (see birfix notes):
# 1. EVENT_SEMAPHORE_RANGE_CLEAR InstISA is rejected ("ISA wrong length").
# 2. Engine instructions only carry one semaphore-wait slot; extra waits are
#    peeled onto NoOp carriers on the same engine (order-preserving).
# ----------------------------------------------------------------------------


def _patched_clear_and_free_semaphores(self, sems):
    if not sems:
        return
    sem_nums = [s.num if hasattr(s, "num") else s for s in sems]
    self._state.prepend_free_semaphores(sem_nums)
    for poison_set in self._tile_sem_poison_stack:
        poison_set.update(sem_nums)


def _fix_bir_waits(bir_json: bytes) -> bytes:
    bir = json.loads(bir_json)
    ctr = 0
    changed = False
    for func in bir.get("functions", []):
        for blk in func.get("blocks", []):
            out = []
            for inst in blk.get("instructions", []):
                si = inst.get("sync_info") or {}
                waits = si.get("on_wait") or []
                if len(waits) > 1:
                    for w in waits[:-1]:
                        ctr += 1
                        out.append(
                            {
                                "debug": inst.get("debug"),
                                "engine": inst.get("engine", "SP"),
                                "ins": [],
                                "name": f"IWF-{ctr}",
                                "opcode": "NoOp",
                                "outs": [],
                                "sync_info": {"on_wait": [w]},
                            }
                        )
                    si = dict(si)
                    si["on_wait"] = waits[-1:]
                    inst = dict(inst)
                    inst["sync_info"] = si
                    changed = True
                out.append(inst)
            blk["instructions"] = out
    return json.dumps(bir).encode() if changed else bir_json


_orig_compile_bir_kernel = _bass_utils.compile_bir_kernel


def _patched_compile_bir_kernel(bir_json, tmpdir, neff_name="file.neff"):
    if isinstance(bir_json, str):
        bir_json = bir_json.encode()
    return _orig_compile_bir_kernel(_fix_bir_waits(bir_json), tmpdir, neff_name)


def _install_patches():
    if getattr(bass.Bass, "_mhsa_patched", False):
        return
    bass.Bass.clear_and_free_semaphores = _patched_clear_and_free_semaphores
    bass.Bass._mhsa_patched = True
    _bass_utils.compile_bir_kernel = _patched_compile_bir_kernel
    try:
        import concourse.bass2jax as _b2j

        _b2j.compile_bir_kernel = _patched_compile_bir_kernel
    except ImportError:
        pass


_install_patches()

# ----------------------------------------------------------------------------
# Problem constants (hardcoded per spec)
# ----------------------------------------------------------------------------
B, T, E, H = 4, 2048, 1024, 16
HD = E // H  # 64
P = 128
NB = T // P  # 16 query/key blocks
NQ = 8  # query blocks per core
EC = E // P  # 8 e-chunks
SCALE = 1.0 / float(np.sqrt(T))
EPS = 1e-6
BF = mybir.dt.bfloat16
F32 = mybir.dt.float32
F8 = mybir.dt.float8e4
NPBF = ml_dtypes.bfloat16
NPF8 = ml_dtypes.float8_e4m3
# fp8 weights are pre-scaled by 16 host-side (keeps N(0, 0.02) entries out
# of the fp8e4m3 subnormal range); the PSUM drain multiplies by 1/16
WSCALE = 16.0
WINV = 1.0 / WSCALE
DR = mybir.MatmulPerfMode.DoubleRow

# query-block assignment: pairs (j, 15-j) so both cores of a batch pair see
# padded lengths {2,4,...,16}; blocks listed in ASCENDING padded length
BLOCKS_A = [0, 2, 4, 6, 9, 11, 13, 15]  # true lengths 1,3,5,7,10,12,14,16
BLOCKS_B = [1, 3, 5, 7, 8, 10, 12, 14]  # true lengths 2,4,6,8,9,11,13,15

# device-side q-block order: DESCENDING padded length; two groups of 4
GLS = {0: (16, 14, 12, 10), 1: (8, 6, 4, 2)}
# mask instances: (group, key block j, boundary q-block index bi); the
# boundary block is always the LAST active block of the prefix at that j
MASK_INST = []
for _g in (0, 1):
    for _j in range(GLS[_g][0]):
        for _bi, _L in enumerate(GLS[_g]):
            if _j in (_L - 2, _L - 1):
                MASK_INST.append((_g, _j, _bi))
MASK_IDX = {(g, j): (idx, bi) for idx, (g, j, bi) in enumerate(MASK_INST)}
assert len(MASK_INST) == 16


def _width(g, j):
    return 128 * sum(1 for L in GLS[g] if L > j)


_nc_cache = {}


def _build_nc():
    if "nc" in _nc_cache:
        return _nc_cache["nc"]
    nc = bass.Bass(num_devices=8)

    # inputs (per-core)
    xT_d = nc.dram_tensor("xT", [E, T], F8, kind="ExternalInput")
    xTq_d = nc.dram_tensor("xTq", [E, NQ * P], BF, kind="ExternalInput")
    xT8q_d = nc.dram_tensor("xT8q", [E, NQ * P], F8, kind="ExternalInput")
    WqT_d = nc.dram_tensor("WqT", [E, E], F8, kind="ExternalInput")
    WkT_d = nc.dram_tensor("WkT", [E, E], F8, kind="ExternalInput")
    WvT_d = nc.dram_tensor("WvT", [E, E], F8, kind="ExternalInput")
    WpT_d = nc.dram_tensor("WpT", [E, E], BF, kind="ExternalInput")
    cF32_d = nc.dram_tensor("cF32", [P, 16], F32, kind="ExternalInput")
    cBF_d = nc.dram_tensor("cBF", [P, 4 * E + 16 * P], BF, kind="ExternalInput")
    y_d = nc.dram_tensor("y", [NQ, P, E], BF, kind="ExternalOutput")

    with tile.TileContext(nc) as tc:
        with ExitStack() as ctx:
            consts = ctx.enter_context(tc.tile_pool(name="consts", bufs=1))
            big = ctx.enter_context(tc.tile_pool(name="big", bufs=1))
            wpool = ctx.enter_context(tc.tile_pool(name="wpool", bufs=1))
            # xT is only needed during the QKV phase; its pool is closed
            # before the attention working set is allocated
            xtp = ctx.enter_context(tc.tile_pool(name="xtp", bufs=1))
            _psA_cm = tc.tile_pool(name="psA", bufs=1, space="PSUM")
            ps = _psA_cm.__enter__()

            def load_w(dram, name, interleave_with=None, split_first=False,
                       dtype=F8):
                # two half-tiles in a 3-slot rotation: the next projection's
                # first half streams in while the previous one's second half
                # is still being consumed. ONE DMA per half (HWDGE issue is a
                # serialized ~625ns/DMA shared resource — minimize count)
                halves = []
                for hf in range(2):
                    w = wpool.tile(
                        [P, EC, E // 2], dtype, tag="wh", bufs=3,
                        name=f"{name}{hf}"
                    )
                    if split_first and hf == 0:
                        # first fb-chunk separately: the first K matmul group
                        # only needs cols 0:128, so it starts ~3us earlier
                        nc.sync.dma_start(
                            w[:, :, 0:P],
                            dram.rearrange("(c p) f -> p c f", p=P)[:, :, 0:P],
                        )
                        if interleave_with is not None:
                            interleave_with(hf)
                        nc.sync.dma_start(
                            w[:, :, P:512],
                            dram.rearrange("(c p) f -> p c f", p=P)[:, :, P:512],
                        )
                    else:
                        nc.sync.dma_start(
                            w[:, :, :],
                            dram.rearrange("(c p) f -> p c f", p=P)[
                                :, :, hf * 512 : (hf + 1) * 512
                            ],
                        )
                        if interleave_with is not None:
                            interleave_with(hf)
                    halves.append(w)
                return halves

            # PE-critical loads first. HWDGE queue order: Wk half0, xT win0,
            # f32 consts (bk needed by the first bias add), Wk half1, then
            # the remaining xT windows — the ts-outer K loop consumes one
            # window per ~13.7us so the serialized DMA stream stays ahead
            xT = xtp.tile([P, EC, T], F8)
            cF32 = consts.tile([P, 16], F32)
            cBF = consts.tile([P, 4 * E + 16 * P], BF)

            def _xt_w(wi):
                nc.sync.dma_start(
                    xT[:, :, wi * 512 : (wi + 1) * 512],
                    xT_d.rearrange("(c p) t -> p c t", p=P)[
                        :, :, wi * 512 : (wi + 1) * 512
                    ],
                )

            def _wk_companion(hf):
                if hf == 0:
                    _xt_w(0)
                    nc.sync.dma_start(cF32[:, :], cF32_d[:, :])

            # serialized-DMA ordering: everything attention-start needs goes
            # first (xT8q for Q, cBF for masks, Wv); the big bf16 xTq is only
            # read by the residual ~150us in, so its DMA is issued after all
            # weight loads (tile allocated here, transfer deferred)
            Wk = load_w(WkT_d, "Wk", interleave_with=_wk_companion, split_first=True)
            _xt_w(1)
            _xt_w(2)
            _xt_w(3)
            xT8q = big.tile([P, EC, NQ * P], F8)
            nc.sync.dma_start(
                xT8q[:, :, :], xT8q_d.rearrange("(c p) t -> p c t", p=P)[:, :, :]
            )
            nc.sync.dma_start(cBF[:, :], cBF_d[:, :])
            xTq = big.tile([P, EC, NQ * P], BF)
            wvp = ctx.enter_context(tc.tile_pool(name="wvp", bufs=1))
            Wv = []
            for hf in range(2):
                wv = wvp.tile([P, EC, E // 2], F8, tag=f"wv{hf}", bufs=1,
                              name=f"Wv{hf}")
                nc.sync.dma_start(
                    wv[:, :, :],
                    WvT_d.rearrange("(c p) f -> p c f", p=P)[
                        :, :, hf * 512 : (hf + 1) * 512
                    ],
                )
                Wv.append(wv)
            # packed-constant layout in cBF: bv | bp | gamma | beta | masks
            OFF_BV, OFF_BP, OFF_G, OFF_B, OFF_M = 0, E, 2 * E, 3 * E, 4 * E

            def mall_at(idx):
                return cBF[:, OFF_M + idx * P : OFF_M + (idx + 1) * P]

            ones64 = consts.tile([P, 64], BF)
            nc.vector.memset(ones64[:], 1.0)

            # persistent intermediates
            KT = big.tile([P, EC, T], BF)  # K^T  [f, t]
            QT = big.tile([P, EC, NQ * P], BF)  # Q^T  [f, t_own]
            # fp8 V and eS enable DoubleRow AV matmuls: the key-block pair
            # is already a free dim in BOTH operands, so no layout remap.
            # Softmax normalization cancels most of the eS quantization
            # (the denominators come from the same quantized weights).
            Vx = big.tile([P, NB, H, HD + 1], F8)  # V ext [t, h, d|1]
            zT = big.tile([P, EC, NQ * P], BF)  # z^T  [e, t_own]
            nc.vector.memset(Vx[:, :, :, HD : HD + 1], 1.0)

            # ---------------- K^T = Wk^T.T-chunks x xT + bk ----------------
            # ts outer: each xT window feeds all 8 fb groups (~13.7us of PE
            # work) so the next window's DMA completes in the shadow
            for ts_ in range(T // 512):
                for fb in range(EC):
                    pk = ps.tile([P, 512], F32, tag="mm512", bufs=6, name="pk")
                    for c in range(0, EC, 2):
                        nc.tensor.matmul(
                            pk[:],
                            Wk[fb // 4][:, c : c + 2, (fb % 4) * P : (fb % 4 + 1) * P],
                            xT[:, c : c + 2, ts_ * 512 : (ts_ + 1) * 512],
                            start=(c == 0),
                            stop=(c == EC - 2),
                            perf_mode=DR,
                        )
                    # drains alternate DVE/ACT (ACT is idle until attention;
                    # K^T partitions are features, so the bias is
                    # per-partition — exactly activation's bias operand).
                    # With fp8 the PE is 4x faster here and a single drain
                    # engine becomes the QKV-phase bottleneck.
                    if fb % 2 == 0:
                        nc.scalar.activation(
                            KT[:, fb, ts_ * 512 : (ts_ + 1) * 512],
                            pk[:],
                            mybir.ActivationFunctionType.Identity,
                            scale=WINV,
                            bias=cF32[:, fb : fb + 1],
                        )
                    else:
                        nc.vector.tensor_scalar(
                            out=KT[:, fb, ts_ * 512 : (ts_ + 1) * 512],
                            in0=pk[:],
                            scalar1=WINV,
                            scalar2=cF32[:, fb : fb + 1],
                            op0=mybir.AluOpType.mult,
                            op1=mybir.AluOpType.add,
                        )

            # ---------------- V (heads 0-7, blocks 0-7) ---------------------
            # V blocks 8-15, V heads 8-15, and Q feature-blocks 4-7 are
            # deferred: their matmul groups interleave into the exp-bound
            # attention wave, keeping PE busy while the Activation engine
            # catches up on exponentials. Wv lives in its own persistent pool
            # because its readers now extend deep into the attention phase
            # (the 3-slot weight rotation would reuse its slots too early).
            def emit_v_group(tb, fs, pool_tag):
                pv = ps.tile([P, 512], F32, tag=pool_tag, bufs=6 if pool_tag == "mm512" else 2, name="pv")
                for c in range(0, EC, 2):
                    nc.tensor.matmul(
                        pv[:],
                        xT[:, c : c + 2, tb * P : (tb + 1) * P],
                        Wv[fs][:, c : c + 2, :],
                        start=(c == 0),
                        stop=(c == EC - 2),
                        perf_mode=DR,
                    )
                # bv is NOT added here: softmax rows sum to 1, so the bias
                # contributes exactly bv per feature after normalization —
                # the host pre-adds it into the residual input xTq instead.
                # Scale-only drain; pre-phase drains alternate onto the
                # still-idle ACT engine.
                if pool_tag == "mm512" and tb % 2 == 0:
                    nc.scalar.activation(
                        Vx[:, tb, fs * 8 : (fs + 1) * 8, 0:HD],
                        pv[:, :].rearrange("p (h d) -> p h d", d=HD),
                        mybir.ActivationFunctionType.Identity,
                        scale=WINV,
                    )
                else:
                    nc.vector.tensor_scalar_mul(
                        Vx[:, tb, fs * 8 : (fs + 1) * 8, 0:HD],
                        pv[:, :].rearrange("p (h d) -> p h d", d=HD),
                        WINV,
                    )

            for tb in range(8):
                emit_v_group(tb, 0, "mm512")

            # ---------------- Q^T (fb 0-3) = Wq^T-chunks x xTq + bq --------
            Wq = load_w(WqT_d, "Wq")

            def emit_q_group(fb, ts_, pool_tag):
                pq = ps.tile([P, 512], F32, tag=pool_tag, bufs=6 if pool_tag == "mm512" else 2, name="pq")
                for c in range(0, EC, 2):
                    nc.tensor.matmul(
                        pq[:],
                        Wq[fb // 4][:, c : c + 2, (fb % 4) * P : (fb % 4 + 1) * P],
                        xT8q[:, c : c + 2, ts_ * 512 : (ts_ + 1) * 512],
                        start=(c == 0),
                        stop=(c == EC - 2),
                        perf_mode=DR,
                    )
                if pool_tag == "mm512" and fb % 2 == 0:
                    # pre-phase only: ACT is idle before attention starts
                    nc.scalar.activation(
                        QT[:, fb, ts_ * 512 : (ts_ + 1) * 512],
                        pq[:],
                        mybir.ActivationFunctionType.Identity,
                        scale=WINV,
                        bias=cF32[:, EC + fb : EC + fb + 1],
                    )
                else:
                    nc.vector.tensor_scalar(
                        out=QT[:, fb, ts_ * 512 : (ts_ + 1) * 512],
                        in0=pq[:],
                        scalar1=WINV,
                        scalar2=cF32[:, EC + fb : EC + fb + 1],
                        op0=mybir.AluOpType.mult,
                        op1=mybir.AluOpType.add,
                    )

            for fb in range(4):
                for ts_ in range(NQ * P // 512):
                    emit_q_group(fb, ts_, "mm512")

            # Wp streams in during attention (3-slot rotation frees Wq slots)
            Wp = load_w(WpT_d, "Wp", dtype=BF)
            # deferred residual input (first read at wave B part 2)
            nc.sync.dma_start(
                xTq[:, :, :], xTq_d.rearrange("(c p) t -> p c t", p=P)[:, :, :]
            )

            # ---------------- attention ----------------
            work = ctx.enter_context(tc.tile_pool(name="work", bufs=2))
            _psA_cm.__exit__(None, None, None)
            _psB_cm = tc.tile_pool(name="psB", bufs=1, space="PSUM")
            ps = _psB_cm.__enter__()

            # unit schedule: wave A = heads 0-7 (g0 then g1), wave B = heads
            # 8-15 g0, residual(0), heads 8-15 g1 with proj tb0-3 interleaved
            units = []
            for h in range(8):
                for g in (0, 1):
                    for p_ in range(GLS[g][0] // 2):
                        units.append((g, h, p_))
            for h in range(8, H):
                for p_ in range(GLS[0][0] // 2):
                    units.append((0, h, p_))
            for h in range(8, H):
                for p_ in range(GLS[1][0] // 2):
                    units.append((1, h, p_))
            WAVE_A_N = 96  # units in wave A
            PART2_AT = WAVE_A_N + 64  # first (g1, h>=8) unit: residual(0) here

            # fillers: V key-blocks 8-15 for heads 0-7 land up front (needed
            # by h0's AV from unit 4 on), then deferred V heads 8-15 and
            # Q fb 4-7 spread through wave A and slightly into wave B part 1;
            # projection tb0-3 spread through part 2
            fillers = {}
            for tb in range(8, NB):
                fillers.setdefault((tb - 8) // 2, []).append(("v0", tb))
            deferred = []
            for tb in range(NB):
                deferred.append(("v", tb))
                if tb < 8:
                    deferred.append(("q", tb))
            for i, d in enumerate(deferred):
                fillers.setdefault(
                    4 + i * (WAVE_A_N + 1) // len(deferred), []
                ).append(d)
            for i in range(4):
                fillers.setdefault(PART2_AT + 6 + 8 * i, []).append(("proj", i))

            def emit_S(u):
                g, h, p_ = units[u]
                j0 = 2 * p_
                w = _width(g, j0)
                hb = (h % 2) * 64
                pS = ps.tile([P, 1024], F32, tag="pS", bufs=2, name="pS")
                for jj in (0, 1):
                    j = j0 + jj
                    nc.tensor.matmul(
                        pS[:, jj * 512 : jj * 512 + w],
                        KT[hb : hb + 64, h // 2, j * P : (j + 1) * P],
                        QT[hb : hb + 64, h // 2, g * 512 : g * 512 + w],
                        start=True,
                        stop=True,
                    )
                return pS

            def emit_division_copy(pO):
                # stage 1, right after the group's last AV: denominators row
                # (accumulated via the Vx ones column) to SBUF. Runs on DVE
                # while the next unit's scores occupy the PE, so stage 2's
                # broadcast matmul doesn't head-of-line block the PE queue.
                # (A broadcast-DMA variant measured slower: the HWDGE+DMA
                # latency in the chain outweighs the saved DVE/PE ops.)
                rr = work.tile([P, 512], BF, tag="rr", bufs=2, name="rr")
                with nc.allow_low_precision(
                    reason="softmax denominators: bf16 is ample (~0.4% on a "
                    "per-query scale factor)"
                ):
                    nc.vector.tensor_copy(rr[64:65, :], pO[64:65, :])
                return rr

            def emit_division(h, g, pO, rr):
                hb = (h % 2) * 64
                # stage 2: broadcast across 64 partitions with a K=1 matmul
                # into the bank's unused upper rows (bf16: a f32 matmul costs
                # 4 cycles/row), reciprocal into SBUF, then one multiply
                # straight into z^T (a DVE op may read at most one PSUM
                # operand)
                with nc.allow_low_precision(
                    reason="softmax denominators: bf16 is ample (~0.4% on a "
                    "per-query scale factor)"
                ):
                    nc.tensor.matmul(
                        pO[64:128, :], ones64[64:65, :], rr[64:65, :],
                        start=True, stop=True,
                    )
                    nc.vector.reciprocal(rr[0:64, :], pO[64:128, :])
                nc.vector.tensor_tensor(
                    out=zT[hb : hb + 64, h // 2, g * 512 : (g + 1) * 512],
                    in0=pO[0:HD, :], in1=rr[0:64, :],
                    op=mybir.AluOpType.mult,
                )

            def emit_residual(g, chunks):
                cols = slice(g * 512, (g + 1) * 512)
                for c in chunks:
                    nc.vector.tensor_tensor(
                        out=zT[:, c, cols], in0=zT[:, c, cols],
                        in1=xTq[:, c, cols], op=mybir.AluOpType.add,
                    )

            inv_e = 1.0 / float(E)

            def emit_proj_tb(tb, last=False):
                # bias-add fused with row-sum accumulation (mean), variance
                # via Square(y - mu) with accum, final normalize as one
                # scale+bias activation; gamma/beta on the idle Pool engine
                # except for the last block (shortest critical chain on DVE)
                y_sb = work.tile([P, E], F32, tag="ysb", bufs=2, name="y_sb")
                s0 = work.tile([P, 1], F32, tag="stat", bufs=16, name="s0")
                ysum = work.tile([P, 1], F32, tag="stat", bufs=16, name="ysum")
                for fs in range(E // 512):
                    py = ps.tile([P, 512], F32, tag="py", bufs=2, name="py")
                    for c in range(EC):
                        nc.tensor.matmul(
                            py[:],
                            zT[:, c, tb * P : (tb + 1) * P],
                            Wp[fs][:, c, :],
                            start=(c == 0),
                            stop=(c == EC - 1),
                        )
                    nc.vector.scalar_tensor_tensor(
                        out=y_sb[:, fs * 512 : (fs + 1) * 512],
                        in0=py[:],
                        scalar=0.0,
                        in1=cBF[:, OFF_BP + fs * 512 : OFF_BP + (fs + 1) * 512],
                        op0=mybir.AluOpType.add,
                        op1=mybir.AluOpType.add,
                        accum_out=(s0 if fs == 0 else ysum)[:, 0:1],
                    )
                negmu = work.tile([P, 1], F32, tag="stat", bufs=16, name="negmu")
                nc.vector.tensor_tensor(
                    out=negmu[:], in0=s0[:], in1=ysum[:], op=mybir.AluOpType.add
                )
                nc.vector.tensor_scalar_mul(negmu[:], negmu[:], -inv_e)
                # ycb doubles as bf16 scratch for the variance pass's unused
                # main output, then holds the normalized result; gamma/beta
                # run in bf16 (4x DVE mode) and y ships as bf16. Variance and
                # normalize stay on ACT: putting them on DVE delays the
                # division multiplies behind them in DVE's in-order queue,
                # which stalls the PE's pO rotation.
                ycb = work.tile([P, E], BF, tag="ycb", bufs=2, name="ycb")
                var = work.tile([P, 1], F32, tag="stat", bufs=16, name="var")
                nc.scalar.activation(
                    ycb[:], y_sb[:], mybir.ActivationFunctionType.Square,
                    bias=negmu[:, 0:1], accum_out=var[:],
                )
                rstd = work.tile([P, 1], F32, tag="stat", bufs=16, name="rstd")
                nc.vector.tensor_scalar(
                    out=rstd[:], in0=var[:], scalar1=inv_e, scalar2=float(EPS),
                    op0=mybir.AluOpType.mult, op1=mybir.AluOpType.add,
                )
                nc.scalar.activation(
                    rstd[:], rstd[:], mybir.ActivationFunctionType.Sqrt
                )
                nc.vector.reciprocal(rstd[:], rstd[:])
                nmr = work.tile([P, 1], F32, tag="stat", bufs=16, name="nmr")
                nc.vector.tensor_tensor(
                    out=nmr[:], in0=negmu[:], in1=rstd[:], op=mybir.AluOpType.mult
                )
                nc.scalar.activation(
                    ycb[:], y_sb[:], mybir.ActivationFunctionType.Identity,
                    scale=rstd[:, 0:1], bias=nmr[:, 0:1],
                )
                nc.vector.tensor_tensor(
                    out=ycb[:], in0=ycb[:], in1=cBF[:, OFF_G : OFF_G + E],
                    op=mybir.AluOpType.mult,
                )
                nc.vector.tensor_tensor(
                    out=ycb[:], in0=ycb[:], in1=cBF[:, OFF_B : OFF_B + E],
                    op=mybir.AluOpType.add,
                )
                nc.sync.dma_start(y_d[tb, :, :], ycb[:])

            pO_cur = None
            pending_divs = []
            prev_S = emit_S(0)
            for u, (g, h, p_) in enumerate(units):
                j0 = 2 * p_
                w = _width(g, j0)
                maxL = GLS[g][0]
                if p_ == 0:
                    if pending_divs:
                        pending_divs.pop(0)()
                    pO_cur = ps.tile([P, 512], F32, tag="pO", bufs=2, name="pO")
                pO = pO_cur
                pS = prev_S
                eS = work.tile([P, 1024], F8, tag="eS", bufs=3, name="eS")
                nc.scalar.activation(
                    eS[:, :].rearrange("p (u q) -> p u q", u=2)[:, :, 0:w],
                    pS[:, :].rearrange("p (u q) -> p u q", u=2)[:, :, 0:w],
                    mybir.ActivationFunctionType.Exp,
                    scale=SCALE,
                )
                if u + 1 < len(units):
                    prev_S = emit_S(u + 1)
                if u == PART2_AT:
                    # all g0 divisions and heads 0-7's g1 divisions are done
                    emit_residual(0, range(EC))
                    emit_residual(1, range(4))
                for kind, arg in fillers.get(u, ()):
                    if kind == "v":
                        emit_v_group(arg, 1, "py")
                    elif kind == "v0":
                        emit_v_group(arg, 0, "py")
                    elif kind == "q":
                        emit_q_group(4 + arg // 2, arg % 2, "py")
                    else:
                        emit_proj_tb(arg)
                mi = MASK_IDX.get((g, j0))
                if mi is not None:
                    # both key blocks of the pair mask the same (last active)
                    # q-block, and their mask instances are consecutive in
                    # cBF: one 3D-AP multiply covers both halves
                    idx, bi = mi
                    assert MASK_IDX[(g, j0 + 1)] == (idx + 1, bi)
                    eSv = eS[:, :].rearrange("p (u q) -> p u q", u=2)[
                        :, :, bi * P : (bi + 1) * P
                    ]
                    nc.vector.tensor_tensor(
                        out=eSv, in0=eSv,
                        in1=cBF[:, OFF_M + idx * P : OFF_M + (idx + 2) * P]
                        .rearrange("p (u q) -> p u q", u=2),
                        op=mybir.AluOpType.mult,
                    )
                nc.tensor.matmul(
                    pO[0 : HD + 1, 0:w],
                    Vx[:, j0 : j0 + 2, h, :],
                    eS[:, :].rearrange("p (v q) -> p v q", v=2)[:, :, 0:w],
                    start=(j0 == 0),
                    stop=(j0 + 1 == maxL - 1),
                    perf_mode=DR,
                    skip_group_check=True,
                )
                if j0 + 1 == maxL - 1:
                    rr_cur = emit_division_copy(pO)

                    def _div(h=h, g=g, pO=pO, rr=rr_cur):
                        emit_division(h, g, pO, rr)
                        if g == 1 and h >= 9 and h % 2 == 1:
                            # z^T feature chunk h//2 complete for both column
                            # groups: add the residual now so the tail
                            # projection's contraction can start early
                            emit_residual(1, [h // 2])

                    pending_divs.append(_div)
            while pending_divs:
                pending_divs.pop(0)()

            # ---------------- tail: residual + projection for group 1 ------
            for tb in range(4, NQ):
                emit_proj_tb(tb, last=(tb == NQ - 1))

            _psB_cm.__exit__(None, None, None)

    _nc_cache["nc"] = nc
    return nc


def _make_mall(ownd):
    """Mask tiles for this core's descending-ordered q-blocks.

    Instance (g, j, bi): multiply eS columns of boundary q-block bi at key
    block j. Pattern depends on whether the block's true length equals the
    padded length (l_true == L) or falls one short (l_true == L-1)."""
    tril_t = (np.arange(P)[:, None] <= np.arange(P)[None, :]).astype(np.float32)
    mall = np.zeros((16, P, P), np.float32)
    for idx, (g, j, bi) in enumerate(MASK_INST):
        L = GLS[g][bi]
        block = ownd[g * 4 + bi]
        l_true = block + 1
        assert l_true in (L, L - 1)
        if j == L - 2:
            mall[idx] = 1.0 if l_true == L else tril_t
        else:
            mall[idx] = tril_t if l_true == L else 0.0
    # device layout [P(k-local), 16, P(q-local)]
    return np.ascontiguousarray(mall.transpose(1, 0, 2)).astype(NPBF)


def kernel(x, Wq, bq, Wk, bk, Wv, bv, Wp, bp, gamma, beta):
    x = np.asarray(x, np.float32)
    nc = _build_nc()

    WqT = np.ascontiguousarray(np.asarray(Wq, np.float32).T * WSCALE).astype(NPF8)
    WkT = np.ascontiguousarray(np.asarray(Wk, np.float32).T * WSCALE).astype(NPF8)
    WvT = np.ascontiguousarray(np.asarray(Wv, np.float32).T * WSCALE).astype(NPF8)
    WpT = np.ascontiguousarray(np.asarray(Wp, np.float32).T).astype(NPBF)
    bqT = np.ascontiguousarray(np.asarray(bq, np.float32).reshape(EC, P).T)
    bkT = np.ascontiguousarray(np.asarray(bk, np.float32).reshape(EC, P).T)
    cF32 = np.concatenate([bkT, bqT], axis=1)  # [P, 16]
    bcast4 = [
        np.broadcast_to(np.asarray(v, np.float32), (P, E))
        for v in (bv, bp, gamma, beta)
    ]
    # descending padded length = reversed block list
    ownd_map = {0: list(reversed(BLOCKS_A)), 1: list(reversed(BLOCKS_B))}
    cBF_map = {
        hh: np.ascontiguousarray(
            np.concatenate(
                bcast4 + [_make_mall(ownd_map[hh]).reshape(P, 16 * P)], axis=1
            )
        ).astype(NPBF)
        for hh in (0, 1)
    }

    in_maps = []
    for core in range(8):
        b, hh = core // 2, core % 2
        ownd = ownd_map[hh]
        own = np.concatenate([np.arange(blk * P, (blk + 1) * P) for blk in ownd])
        xb = x[b]  # (T, E)
        xT = np.ascontiguousarray(xb.T).astype(NPF8)
        # bv folded in: each head's output picks up exactly +bv after the
        # softmax division (weights sum to 1), so it lands here instead
        xTq = np.ascontiguousarray(
            xb[own].T + np.asarray(bv, np.float32)[:, None]
        ).astype(NPBF)
        xT8q = np.ascontiguousarray(xb[own].T).astype(NPF8)
        in_maps.append(
            {
                "xT": xT,
                "xTq": xTq,
                "xT8q": xT8q,
                "WqT": WqT,
                "WkT": WkT,
                "WvT": WvT,
                "WpT": WpT,
                "cF32": cF32,
                "cBF": cBF_map[hh],
            }
        )

    import os

    trace = bool(int(os.environ.get("MHSA_TRACE", "0")))
    res = run_bass_kernel_spmd(
        nc, in_maps, core_ids=list(range(8)), trace=trace,
        trace_cores=list(range(8)) if trace else None,
    )
    if trace and res.exec_time_ns is not None:
        print(f"HW exec time: {res.exec_time_ns} ns")
        if res.mean_exec_time_ns is not None:
            print(f"HW exec mean across cores: {res.mean_exec_time_ns:.0f} ns")
        kernel.last_exec_time_ns = res.exec_time_ns
        kernel.last_trace = res.instructions_and_trace

    out = np.empty((B, T, E), np.float32)
    for core in range(8):
        b, hh = core // 2, core % 2
        ownd = ownd_map[hh]
        y = res.results[core]["y"]  # (NQ, P, E) bf16
        for k, blk in enumerate(ownd):
            out[b, blk * P : (blk + 1) * P, :] = y[k].astype(np.float32)
    return out


# revision 113
# speedup vs baseline: 1.0517x; 1.0203x over previous
"""Multi-head self-attention (B=4, T=2048, E=1024, H=16) on 8 trn2 NeuronCores.

Sharding: core (b, h) = batch b, token-half h. Each core computes K/V for the
full sequence (duplicated within the batch pair), Q for its own 8 query blocks
of 128 tokens, causal attention for those blocks, then the output projection
and LayerNorm for its own tokens.

Attention restructure (vs the 128-wide-per-head-pair baseline): each core's
query blocks are ordered by DESCENDING padded causal length (16,14,12,10 |
8,6,4,2 key blocks), so for key block j the active query blocks form a
contiguous prefix. Scores/AV run one matmul per (head, group-of-4-q-blocks,
key block) with free dim up to 512, cutting PE instruction count ~3x. The
softmax denominator division runs once per (head, group) on 512 columns.
Projection+LN for the first 4 token blocks is interleaved into the second
(light) attention group to shrink the tail.

Causal balance: query blocks are paired (j, 15-j) so both cores of a batch
process blocks with padded key-lengths 2,4,...,16; host-supplied mask tiles
encode the true causal structure, keeping the compiled program identical
across cores (SPMD).

All matmuls run in bf16 with fp32 PSUM accumulation (validated ~2e-3
scale-relative error vs the fp32 reference).
"""
import json
import numpy as np
import ml_dtypes
from contextlib import ExitStack

import concourse.bass as bass
import concourse.bass_utils as _bass_utils
import concourse.tile as tile
from concourse import mybir
from concourse.bass_utils import run_bass_kernel_spmd

# ----------------------------------------------------------------------------
# Toolchain workarounds for this container's walrus build (see birfix notes):
# 1. EVENT_SEMAPHORE_RANGE_CLEAR InstISA is rejected ("ISA wrong length").
# 2. Engine instructions only carry one semaphore-wait slot; extra waits are
#    peeled onto NoOp carriers on the same engine (order-preserving).
# ----------------------------------------------------------------------------


def _patched_clear_and_free_semaphores(self, sems):
    if not sems:
        return
    sem_nums = [s.num if hasattr(s, "num") else s for s in sems]
    self._state.prepend_free_semaphores(sem_nums)
    for poison_set in self._tile_sem_poison_stack:
        poison_set.update(sem_nums)


def _fix_bir_waits(bir_json: bytes) -> bytes:
    bir = json.loads(bir_json)
    ctr = 0
    changed = False
    for func in bir.get("functions", []):
        for blk in func.get("blocks", []):
            out = []
            for inst in blk.get("instructions", []):
                si = inst.get("sync_info") or {}
                waits = si.get("on_wait") or []
                if len(waits) > 1:
                    for w in waits[:-1]:
                        ctr += 1
                        out.append(
                            {
                                "debug": inst.get("debug"),
                                "engine": inst.get("engine", "SP"),
                                "ins": [],
                                "name": f"IWF-{ctr}",
                                "opcode": "NoOp",
                                "outs": [],
                                "sync_info": {"on_wait": [w]},
                            }
                        )
                    si = dict(si)
                    si["on_wait"] = waits[-1:]
                    inst = dict(inst)
                    inst["sync_info"] = si
                    changed = True
                out.append(inst)
            blk["instructions"] = out
    return json.dumps(bir).encode() if changed else bir_json


_orig_compile_bir_kernel = _bass_utils.compile_bir_kernel


def _patched_compile_bir_kernel(bir_json, tmpdir, neff_name="file.neff"):
    if isinstance(bir_json, str):
        bir_json = bir_json.encode()
    return _orig_compile_bir_kernel(_fix_bir_waits(bir_json), tmpdir, neff_name)


def _install_patches():
    if getattr(bass.Bass, "_mhsa_patched", False):
        return
    bass.Bass.clear_and_free_semaphores = _patched_clear_and_free_semaphores
    bass.Bass._mhsa_patched = True
    _bass_utils.compile_bir_kernel = _patched_compile_bir_kernel
    try:
        import concourse.bass2jax as _b2j

        _b2j.compile_bir_kernel = _patched_compile_bir_kernel
    except ImportError:
        pass


_install_patches()

# ----------------------------------------------------------------------------
# Problem constants (hardcoded per spec)
# ----------------------------------------------------------------------------
B, T, E, H = 4, 2048, 1024, 16
HD = E // H  # 64
P = 128
NB = T // P  # 16 query/key blocks
NQ = 8  # query blocks per core
EC = E // P  # 8 e-chunks
SCALE = 1.0 / float(np.sqrt(T))
EPS = 1e-6
BF = mybir.dt.bfloat16
F32 = mybir.dt.float32
F8 = mybir.dt.float8e4
NPBF = ml_dtypes.bfloat16
NPF8 = ml_dtypes.float8_e4m3
# fp8 weights are pre-scaled by 16 host-side (keeps N(0, 0.02) entries out
# of the fp8e4m3 subnormal range); the PSUM drain multiplies by 1/16
WSCALE = 16.0
WINV = 1.0 / WSCALE
DR = mybir.MatmulPerfMode.DoubleRow

# query-block assignment: pairs (j, 15-j) so both cores of a batch pair see
# padded lengths {2,4,...,16}; blocks listed in ASCENDING padded length
BLOCKS_A = [0, 2, 4, 6, 9, 11, 13, 15]  # true lengths 1,3,5,7,10,12,14,16
BLOCKS_B = [1, 3, 5, 7, 8, 10, 12, 14]  # true lengths 2,4,6,8,9,11,13,15

# device-side q-block order: DESCENDING padded length; two groups of 4
GLS = {0: (16, 14, 12, 10), 1: (8, 6, 4, 2)}
# mask instances: (group, key block j, boundary q-block index bi); the
# boundary block is always the LAST active block of the prefix at that j
MASK_INST = []
for _g in (0, 1):
    for _j in range(GLS[_g][0]):
        for _bi, _L in enumerate(GLS[_g]):
            if _j in (_L - 2, _L - 1):
                MASK_INST.append((_g, _j, _bi))
MASK_IDX = {(g, j): (idx, bi) for idx, (g, j, bi) in enumerate(MASK_INST)}
assert len(MASK_INST) == 16


def _width(g, j):
    return 128 * sum(1 for L in GLS[g] if L > j)


_nc_cache = {}


def _build_nc():
    if "nc" in _nc_cache:
        return _nc_cache["nc"]
    nc = bass.Bass(num_devices=8)

    # inputs (per-core)
    xT_d = nc.dram_tensor("xT", [E, T], F8, kind="ExternalInput")
    xTq_d = nc.dram_tensor("xTq", [E, NQ * P], BF, kind="ExternalInput")
    xT8q_d = nc.dram_tensor("xT8q", [E, NQ * P], F8, kind="ExternalInput")
    WqT_d = nc.dram_tensor("WqT", [E, E], F8, kind="ExternalInput")
    WkT_d = nc.dram_tensor("WkT", [E, E], F8, kind="ExternalInput")
    WvT_d = nc.dram_tensor("WvT", [E, E], F8, kind="ExternalInput")
    WpT_d = nc.dram_tensor("WpT", [E, E], BF, kind="ExternalInput")
    cF32_d = nc.dram_tensor("cF32", [P, 16], F32, kind="ExternalInput")
    cBF_d = nc.dram_tensor("cBF", [P, 4 * E + 16 * P], BF, kind="ExternalInput")
    y_d = nc.dram_tensor("y", [NQ, P, E], BF, kind="ExternalOutput")

    with tile.TileContext(nc) as tc:
        with ExitStack() as ctx:
            consts = ctx.enter_context(tc.tile_pool(name="consts", bufs=1))
            big = ctx.enter_context(tc.tile_pool(name="big", bufs=1))
            wpool = ctx.enter_context(tc.tile_pool(name="wpool", bufs=1))
            # xT is only needed during the QKV phase; its pool is closed
            # before the attention working set is allocated
            xtp = ctx.enter_context(tc.tile_pool(name="xtp", bufs=1))
            _psA_cm = tc.tile_pool(name="psA", bufs=1, space="PSUM")
            ps = _psA_cm.__enter__()

            def load_w(dram, name, interleave_with=None, split_first=False,
                       dtype=F8):
                # two half-tiles in a 3-slot rotation: the next projection's
                # first half streams in while the previous one's second half
                # is still being consumed. ONE DMA per half (HWDGE issue is a
                # serialized ~625ns/DMA shared resource — minimize count)
                halves = []
                for hf in range(2):
                    w = wpool.tile(
                        [P, EC, E // 2], dtype, tag="wh", bufs=3,
                        name=f"{name}{hf}"
                    )
                    if split_first and hf == 0:
                        # first fb-chunk separately: the first K matmul group
                        # only needs cols 0:128, so it starts ~3us earlier
                        nc.sync.dma_start(
                            w[:, :, 0:P],
                            dram.rearrange("(c p) f -> p c f", p=P)[:, :, 0:P],
                        )
                        if interleave_with is not None:
                            interleave_with(hf)
                        nc.sync.dma_start(
                            w[:, :, P:512],
                            dram.rearrange("(c p) f -> p c f", p=P)[:, :, P:512],
                        )
                    else:
                        nc.sync.dma_start(
                            w[:, :, :],
                            dram.rearrange("(c p) f -> p c f", p=P)[
                                :, :, hf * 512 : (hf + 1) * 512
                            ],
                        )
                        if interleave_with is not None:
                            interleave_with(hf)
                    halves.append(w)
                return halves

            # PE-critical loads first. HWDGE queue order: Wk half0, xT win0,
            # f32 consts (bk needed by the first bias add), Wk half1, then
            # the remaining xT windows — the ts-outer K loop consumes one
            # window per ~13.7us so the serialized DMA stream stays ahead
            xT = xtp.tile([P, EC, T], F8)
            cF32 = consts.tile([P, 16], F32)
            cBF = consts.tile([P, 4 * E + 16 * P], BF)

            def _xt_w(wi):
                nc.sync.dma_start(
                    xT[:, :, wi * 512 : (wi + 1) * 512],
                    xT_d.rearrange("(c p) t -> p c t", p=P)[
                        :, :, wi * 512 : (wi + 1) * 512
                    ],
                )

            def _wk_companion(hf):
                if hf == 0:
                    _xt_w(0)
                    nc.sync.dma_start(cF32[:, :], cF32_d[:, :])

            # serialized-DMA ordering: everything attention-start needs goes
            # first (xT8q for Q, cBF for masks, Wv); the big bf16 xTq is only
            # read by the residual ~150us in, so its DMA is issued after all
            # weight loads (tile allocated here, transfer deferred)
            Wk = load_w(WkT_d, "Wk", interleave_with=_wk_companion, split_first=True)
            _xt_w(1)
            _xt_w(2)
            _xt_w(3)
            xT8q = big.tile([P, EC, NQ * P], F8)
            nc.sync.dma_start(
                xT8q[:, :, :], xT8q_d.rearrange("(c p) t -> p c t", p=P)[:, :, :]
            )
            nc.sync.dma_start(cBF[:, :], cBF_d[:, :])
            xTq = big.tile([P, EC, NQ * P], BF)
            wvp = ctx.enter_context(tc.tile_pool(name="wvp", bufs=1))
            Wv = []
            for hf in range(2):
                wv = wvp.tile([P, EC, E // 2], F8, tag=f"wv{hf}", bufs=1,
                              name=f"Wv{hf}")
                nc.sync.dma_start(
                    wv[:, :, :],
                    WvT_d.rearrange("(c p) f -> p c f", p=P)[
                        :, :, hf * 512 : (hf + 1) * 512
                    ],
                )
                Wv.append(wv)
            # packed-constant layout in cBF: bv | bp | gamma | beta | masks
            OFF_BV, OFF_BP, OFF_G, OFF_B, OFF_M = 0, E, 2 * E, 3 * E, 4 * E

            def mall_at(idx):
                return cBF[:, OFF_M + idx * P : OFF_M + (idx + 1) * P]

            ones64 = consts.tile([P, 64], BF)
            nc.vector.memset(ones64[:], 1.0)

            # persistent intermediates
            # K^T/Q^T in fp8 with host-permuted feature order: head h's hd
            # dims live at partitions (h%4)*32..+32 across chunk-slot pair
            # (2*(h//4), +1) — the chunk is a free dim, so score matmuls
            # run DoubleRow with no device-side remap (the projection's
            # output partition order follows Wk/Wq column order, which the
            # host controls for free).
            KT = big.tile([P, EC, T], F8)  # K^T  [f-perm, t]
            QT = big.tile([P, EC, NQ * P], F8)  # Q^T  [f-perm, t_own]
            # fp8 V and eS enable DoubleRow AV matmuls: the key-block pair
            # is already a free dim in BOTH operands, so no layout remap.
            # Softmax normalization cancels most of the eS quantization
            # (the denominators come from the same quantized weights).
            Vx = big.tile([P, NB, H, HD + 1], F8)  # V ext [t, h, d|1]
            zT = big.tile([P, EC, NQ * P], BF)  # z^T  [e, t_own]
            nc.vector.memset(Vx[:, :, :, HD : HD + 1], 1.0)

            # ---------------- K^T = Wk^T.T-chunks x xT + bk ----------------
            # ts outer: each xT window feeds all 8 fb groups (~13.7us of PE
            # work) so the next window's DMA completes in the shadow
            for ts_ in range(T // 512):
                for fb in range(EC):
                    pk = ps.tile([P, 512], F32, tag="mm512", bufs=6, name="pk")
                    for c in range(0, EC, 2):
                        nc.tensor.matmul(
                            pk[:],
                            Wk[fb // 4][:, c : c + 2, (fb % 4) * P : (fb % 4 + 1) * P],
                            xT[:, c : c + 2, ts_ * 512 : (ts_ + 1) * 512],
                            start=(c == 0),
                            stop=(c == EC - 2),
                            perf_mode=DR,
                        )
                    # drains alternate DVE/ACT (ACT is idle until attention;
                    # K^T partitions are features, so the bias is
                    # per-partition — exactly activation's bias operand).
                    # With fp8 the PE is 4x faster here and a single drain
                    # engine becomes the QKV-phase bottleneck.
                    if fb % 2 == 0:
                        nc.scalar.activation(
                            KT[:, fb, ts_ * 512 : (ts_ + 1) * 512],
                            pk[:],
                            mybir.ActivationFunctionType.Identity,
                            scale=WINV,
                            bias=cF32[:, fb : fb + 1],
                        )
                    else:
                        nc.vector.tensor_scalar(
                            out=KT[:, fb, ts_ * 512 : (ts_ + 1) * 512],
                            in0=pk[:],
                            scalar1=WINV,
                            scalar2=cF32[:, fb : fb + 1],
                            op0=mybir.AluOpType.mult,
                            op1=mybir.AluOpType.add,
                        )

            # ---------------- V (heads 0-7, blocks 0-7) ---------------------
            # V blocks 8-15, V heads 8-15, and Q feature-blocks 4-7 are
            # deferred: their matmul groups interleave into the exp-bound
            # attention wave, keeping PE busy while the Activation engine
            # catches up on exponentials. Wv lives in its own persistent pool
            # because its readers now extend deep into the attention phase
            # (the 3-slot weight rotation would reuse its slots too early).
            def emit_v_group(tb, fs, pool_tag):
                pv = ps.tile([P, 512], F32, tag=pool_tag, bufs=6 if pool_tag == "mm512" else 2, name="pv")
                for c in range(0, EC, 2):
                    nc.tensor.matmul(
                        pv[:],
                        xT[:, c : c + 2, tb * P : (tb + 1) * P],
                        Wv[fs][:, c : c + 2, :],
                        start=(c == 0),
                        stop=(c == EC - 2),
                        perf_mode=DR,
                    )
                # bv is NOT added here: softmax rows sum to 1, so the bias
                # contributes exactly bv per feature after normalization —
                # the host pre-adds it into the residual input xTq instead.
                # Scale-only drain; pre-phase drains alternate onto the
                # still-idle ACT engine.
                if pool_tag == "mm512" and tb % 2 == 0:
                    nc.scalar.activation(
                        Vx[:, tb, fs * 8 : (fs + 1) * 8, 0:HD],
                        pv[:, :].rearrange("p (h d) -> p h d", d=HD),
                        mybir.ActivationFunctionType.Identity,
                        scale=WINV,
                    )
                else:
                    nc.vector.tensor_scalar_mul(
                        Vx[:, tb, fs * 8 : (fs + 1) * 8, 0:HD],
                        pv[:, :].rearrange("p (h d) -> p h d", d=HD),
                        WINV,
                    )

            for tb in range(8):
                emit_v_group(tb, 0, "mm512")

            # ---------------- Q^T (fb 0-3) = Wq^T-chunks x xTq + bq --------
            Wq = load_w(WqT_d, "Wq")

            def emit_q_group(fb, ts_, pool_tag):
                pq = ps.tile([P, 512], F32, tag=pool_tag, bufs=6 if pool_tag == "mm512" else 2, name="pq")
                for c in range(0, EC, 2):
                    nc.tensor.matmul(
                        pq[:],
                        Wq[fb // 4][:, c : c + 2, (fb % 4) * P : (fb % 4 + 1) * P],
                        xT8q[:, c : c + 2, ts_ * 512 : (ts_ + 1) * 512],
                        start=(c == 0),
                        stop=(c == EC - 2),
                        perf_mode=DR,
                    )
                if pool_tag == "mm512" and fb % 2 == 0:
                    # pre-phase only: ACT is idle before attention starts
                    nc.scalar.activation(
                        QT[:, fb, ts_ * 512 : (ts_ + 1) * 512],
                        pq[:],
                        mybir.ActivationFunctionType.Identity,
                        scale=WINV,
                        bias=cF32[:, EC + fb : EC + fb + 1],
                    )
                else:
                    nc.vector.tensor_scalar(
                        out=QT[:, fb, ts_ * 512 : (ts_ + 1) * 512],
                        in0=pq[:],
                        scalar1=WINV,
                        scalar2=cF32[:, EC + fb : EC + fb + 1],
                        op0=mybir.AluOpType.mult,
                        op1=mybir.AluOpType.add,
                    )

            for fb in range(4):
                for ts_ in range(NQ * P // 512):
                    emit_q_group(fb, ts_, "mm512")

            # Wp streams in during attention (3-slot rotation frees Wq slots)
            Wp = load_w(WpT_d, "Wp", dtype=BF)
            # deferred residual input (first read at wave B part 2)
            nc.sync.dma_start(
                xTq[:, :, :], xTq_d.rearrange("(c p) t -> p c t", p=P)[:, :, :]
            )

            # ---------------- attention ----------------
            work = ctx.enter_context(tc.tile_pool(name="work", bufs=2))
            _psA_cm.__exit__(None, None, None)
            _psB_cm = tc.tile_pool(name="psB", bufs=1, space="PSUM")
            ps = _psB_cm.__enter__()

            # unit schedule: wave A = heads 0-7 (g0 then g1), wave B = heads
            # 8-15 g0, residual(0), heads 8-15 g1 with proj tb0-3 interleaved
            units = []
            for h in range(8):
                for g in (0, 1):
                    for p_ in range(GLS[g][0] // 2):
                        units.append((g, h, p_))
            for h in range(8, H):
                for p_ in range(GLS[0][0] // 2):
                    units.append((0, h, p_))
            for h in range(8, H):
                for p_ in range(GLS[1][0] // 2):
                    units.append((1, h, p_))
            WAVE_A_N = 96  # units in wave A
            PART2_AT = WAVE_A_N + 64  # first (g1, h>=8) unit: residual(0) here

            # fillers: V key-blocks 8-15 for heads 0-7 land up front (needed
            # by h0's AV from unit 4 on), then deferred V heads 8-15 and
            # Q fb 4-7 spread through wave A and slightly into wave B part 1;
            # projection tb0-3 spread through part 2
            fillers = {}
            for tb in range(8, NB):
                fillers.setdefault((tb - 8) // 2, []).append(("v0", tb))
            deferred = []
            for tb in range(NB):
                deferred.append(("v", tb))
                if tb < 8:
                    deferred.append(("q", tb))
            for i, d in enumerate(deferred):
                fillers.setdefault(
                    4 + i * (WAVE_A_N + 1) // len(deferred), []
                ).append(d)
            for i in range(4):
                fillers.setdefault(PART2_AT + 6 + 8 * i, []).append(("proj", i))

            def emit_S(u):
                g, h, p_ = units[u]
                j0 = 2 * p_
                w = _width(g, j0)
                hb = (h % 4) * 32
                cs = 2 * (h // 4)
                pS = ps.tile([P, 1024], F32, tag="pS", bufs=2, name="pS")
                for jj in (0, 1):
                    j = j0 + jj
                    nc.tensor.matmul(
                        pS[:, jj * 512 : jj * 512 + w],
                        KT[hb : hb + 32, cs : cs + 2, j * P : (j + 1) * P],
                        QT[hb : hb + 32, cs : cs + 2, g * 512 : g * 512 + w],
                        start=True,
                        stop=True,
                        perf_mode=DR,
                        tile_position=(hb, 0),
                    )
                return pS

            def emit_division_copy(pO):
                # stage 1, right after the group's last AV: denominators row
                # (accumulated via the Vx ones column) to SBUF. Runs on DVE
                # while the next unit's scores occupy the PE, so stage 2's
                # broadcast matmul doesn't head-of-line block the PE queue.
                # (A broadcast-DMA variant measured slower: the HWDGE+DMA
                # latency in the chain outweighs the saved DVE/PE ops.)
                rr = work.tile([P, 512], BF, tag="rr", bufs=2, name="rr")
                with nc.allow_low_precision(
                    reason="softmax denominators: bf16 is ample (~0.4% on a "
                    "per-query scale factor)"
                ):
                    nc.vector.tensor_copy(rr[64:65, :], pO[64:65, :])
                return rr

            def emit_division(h, g, pO, rr):
                hb = (h % 2) * 64
                # stage 2: broadcast across 64 partitions with a K=1 matmul
                # into the bank's unused upper rows (bf16: a f32 matmul costs
                # 4 cycles/row), reciprocal into SBUF, then one multiply
                # straight into z^T (a DVE op may read at most one PSUM
                # operand)
                with nc.allow_low_precision(
                    reason="softmax denominators: bf16 is ample (~0.4% on a "
                    "per-query scale factor)"
                ):
                    nc.tensor.matmul(
                        pO[64:128, :], ones64[64:65, :], rr[64:65, :],
                        start=True, stop=True,
                    )
                    nc.vector.reciprocal(rr[0:64, :], pO[64:128, :])
                nc.vector.tensor_tensor(
                    out=zT[hb : hb + 64, h // 2, g * 512 : (g + 1) * 512],
                    in0=pO[0:HD, :], in1=rr[0:64, :],
                    op=mybir.AluOpType.mult,
                )

            def emit_residual(g, chunks):
                cols = slice(g * 512, (g + 1) * 512)
                for c in chunks:
                    nc.vector.tensor_tensor(
                        out=zT[:, c, cols], in0=zT[:, c, cols],
                        in1=xTq[:, c, cols], op=mybir.AluOpType.add,
                    )

            inv_e = 1.0 / float(E)

            def emit_proj_tb(tb, last=False):
                # bias-add fused with row-sum accumulation (mean), variance
                # via Square(y - mu) with accum, final normalize as one
                # scale+bias activation; gamma/beta on the idle Pool engine
                # except for the last block (shortest critical chain on DVE)
                y_sb = work.tile([P, E], F32, tag="ysb", bufs=2, name="y_sb")
                s0 = work.tile([P, 1], F32, tag="stat", bufs=16, name="s0")
                ysum = work.tile([P, 1], F32, tag="stat", bufs=16, name="ysum")
                for fs in range(E // 512):
                    py = ps.tile([P, 512], F32, tag="py", bufs=2, name="py")
                    for c in range(EC):
                        nc.tensor.matmul(
                            py[:],
                            zT[:, c, tb * P : (tb + 1) * P],
                            Wp[fs][:, c, :],
                            start=(c == 0),
                            stop=(c == EC - 1),
                        )
                    nc.vector.scalar_tensor_tensor(
                        out=y_sb[:, fs * 512 : (fs + 1) * 512],
                        in0=py[:],
                        scalar=0.0,
                        in1=cBF[:, OFF_BP + fs * 512 : OFF_BP + (fs + 1) * 512],
                        op0=mybir.AluOpType.add,
                        op1=mybir.AluOpType.add,
                        accum_out=(s0 if fs == 0 else ysum)[:, 0:1],
                    )
                negmu = work.tile([P, 1], F32, tag="stat", bufs=16, name="negmu")
                nc.vector.tensor_tensor(
                    out=negmu[:], in0=s0[:], in1=ysum[:], op=mybir.AluOpType.add
                )
                nc.vector.tensor_scalar_mul(negmu[:], negmu[:], -inv_e)
                # ycb doubles as bf16 scratch for the variance pass's unused
                # main output, then holds the normalized result; gamma/beta
                # run in bf16 (4x DVE mode) and y ships as bf16. Variance and
                # normalize stay on ACT: putting them on DVE delays the
                # division multiplies behind them in DVE's in-order queue,
                # which stalls the PE's pO rotation.
                ycb = work.tile([P, E], BF, tag="ycb", bufs=2, name="ycb")
                var = work.tile([P, 1], F32, tag="stat", bufs=16, name="var")
                nc.scalar.activation(
                    ycb[:], y_sb[:], mybir.ActivationFunctionType.Square,
                    bias=negmu[:, 0:1], accum_out=var[:],
                )
                rstd = work.tile([P, 1], F32, tag="stat", bufs=16, name="rstd")
                nc.vector.tensor_scalar(
                    out=rstd[:], in0=var[:], scalar1=inv_e, scalar2=float(EPS),
                    op0=mybir.AluOpType.mult, op1=mybir.AluOpType.add,
                )
                nc.scalar.activation(
                    rstd[:], rstd[:], mybir.ActivationFunctionType.Sqrt
                )
                nc.vector.reciprocal(rstd[:], rstd[:])
                nmr = work.tile([P, 1], F32, tag="stat", bufs=16, name="nmr")
                nc.vector.tensor_tensor(
                    out=nmr[:], in0=negmu[:], in1=rstd[:], op=mybir.AluOpType.mult
                )
                nc.scalar.activation(
                    ycb[:], y_sb[:], mybir.ActivationFunctionType.Identity,
                    scale=rstd[:, 0:1], bias=nmr[:, 0:1],
                )
                nc.vector.tensor_tensor(
                    out=ycb[:], in0=ycb[:], in1=cBF[:, OFF_G : OFF_G + E],
                    op=mybir.AluOpType.mult,
                )
                nc.vector.tensor_tensor(
                    out=ycb[:], in0=ycb[:], in1=cBF[:, OFF_B : OFF_B + E],
                    op=mybir.AluOpType.add,
                )
                nc.sync.dma_start(y_d[tb, :, :], ycb[:])

            pO_cur = None
            pending_divs = []
            prev_S = emit_S(0)
            for u, (g, h, p_) in enumerate(units):
                j0 = 2 * p_
                w = _width(g, j0)
                maxL = GLS[g][0]
                if p_ == 0:
                    if pending_divs:
                        pending_divs.pop(0)()
                    pO_cur = ps.tile([P, 512], F32, tag="pO", bufs=2, name="pO")
                pO = pO_cur
                pS = prev_S
                eS = work.tile([P, 1024], F8, tag="eS", bufs=3, name="eS")
                nc.scalar.activation(
                    eS[:, :].rearrange("p (u q) -> p u q", u=2)[:, :, 0:w],
                    pS[:, :].rearrange("p (u q) -> p u q", u=2)[:, :, 0:w],
                    mybir.ActivationFunctionType.Exp,
                    scale=SCALE,
                )
                if u + 1 < len(units):
                    prev_S = emit_S(u + 1)
                if u == PART2_AT:
                    # all g0 divisions and heads 0-7's g1 divisions are done
                    emit_residual(0, range(EC))
                    emit_residual(1, range(4))
                for kind, arg in fillers.get(u, ()):
                    if kind == "v":
                        emit_v_group(arg, 1, "py")
                    elif kind == "v0":
                        emit_v_group(arg, 0, "py")
                    elif kind == "q":
                        emit_q_group(4 + arg // 2, arg % 2, "py")
                    else:
                        emit_proj_tb(arg)
                mi = MASK_IDX.get((g, j0))
                if mi is not None:
                    # both key blocks of the pair mask the same (last active)
                    # q-block, and their mask instances are consecutive in
                    # cBF: one 3D-AP multiply covers both halves
                    idx, bi = mi
                    assert MASK_IDX[(g, j0 + 1)] == (idx + 1, bi)
                    eSv = eS[:, :].rearrange("p (u q) -> p u q", u=2)[
                        :, :, bi * P : (bi + 1) * P
                    ]
                    nc.vector.tensor_tensor(
                        out=eSv, in0=eSv,
                        in1=cBF[:, OFF_M + idx * P : OFF_M + (idx + 2) * P]
                        .rearrange("p (u q) -> p u q", u=2),
                        op=mybir.AluOpType.mult,
                    )
                nc.tensor.matmul(
                    pO[0 : HD + 1, 0:w],
                    Vx[:, j0 : j0 + 2, h, :],
                    eS[:, :].rearrange("p (v q) -> p v q", v=2)[:, :, 0:w],
                    start=(j0 == 0),
                    stop=(j0 + 1 == maxL - 1),
                    perf_mode=DR,
                    skip_group_check=True,
                )
                if j0 + 1 == maxL - 1:
                    rr_cur = emit_division_copy(pO)

                    def _div(h=h, g=g, pO=pO, rr=rr_cur):
                        emit_division(h, g, pO, rr)
                        if g == 1 and h >= 9 and h % 2 == 1:
                            # z^T feature chunk h//2 complete for both column
                            # groups: add the residual now so the tail
                            # projection's contraction can start early
                            emit_residual(1, [h // 2])

                    pending_divs.append(_div)
            while pending_divs:
                pending_divs.pop(0)()

            # ---------------- tail: residual + projection for group 1 ------
            for tb in range(4, NQ):
                emit_proj_tb(tb, last=(tb == NQ - 1))

            _psB_cm.__exit__(None, None, None)

    _nc_cache["nc"] = nc
    return nc


def _make_mall(ownd):
    """Mask tiles for this core's descending-ordered q-blocks.

    Instance (g, j, bi): multiply eS columns of boundary q-block bi at key
    block j. Pattern depends on whether the block's true length equals the
    padded length (l_true == L) or falls one short (l_true == L-1)."""
    tril_t = (np.arange(P)[:, None] <= np.arange(P)[None, :]).astype(np.float32)
    mall = np.zeros((16, P, P), np.float32)
    for idx, (g, j, bi) in enumerate(MASK_INST):
        L = GLS[g][bi]
        block = ownd[g * 4 + bi]
        l_true = block + 1
        assert l_true in (L, L - 1)
        if j == L - 2:
            mall[idx] = 1.0 if l_true == L else tril_t
        else:
            mall[idx] = tril_t if l_true == L else 0.0
    # device layout [P(k-local), 16, P(q-local)]
    return np.ascontiguousarray(mall.transpose(1, 0, 2)).astype(NPBF)


def kernel(x, Wq, bq, Wk, bk, Wv, bv, Wp, bp, gamma, beta):
    x = np.asarray(x, np.float32)
    nc = _build_nc()

    # K/Q feature permutation for DoubleRow scores: slot s, partition p ->
    # feature (4*(s//2) + p//32)*64 + (s%2)*32 + (p%32); the projection's
    # output partition order follows the weight column order, so this costs
    # nothing on-device
    _s = np.arange(E) // P
    _p = np.arange(E) % P
    KQPERM = (4 * (_s // 2) + _p // 32) * 64 + (_s % 2) * 32 + (_p % 32)
    WqT = np.ascontiguousarray(
        np.asarray(Wq, np.float32).T[:, KQPERM] * WSCALE
    ).astype(NPF8)
    WkT = np.ascontiguousarray(
        np.asarray(Wk, np.float32).T[:, KQPERM] * WSCALE
    ).astype(NPF8)
    WvT = np.ascontiguousarray(np.asarray(Wv, np.float32).T * WSCALE).astype(NPF8)
    WpT = np.ascontiguousarray(np.asarray(Wp, np.float32).T).astype(NPBF)
    bqT = np.ascontiguousarray(
        np.asarray(bq, np.float32)[KQPERM].reshape(EC, P).T
    )
    bkT = np.ascontiguousarray(
        np.asarray(bk, np.float32)[KQPERM].reshape(EC, P).T
    )
    cF32 = np.concatenate([bkT, bqT], axis=1)  # [P, 16]
    bcast4 = [
        np.broadcast_to(np.asarray(v, np.float32), (P, E))
        for v in (bv, bp, gamma, beta)
    ]
    # descending padded length = reversed block list
    ownd_map = {0: list(reversed(BLOCKS_A)), 1: list(reversed(BLOCKS_B))}
    cBF_map = {
        hh: np.ascontiguousarray(
            np.concatenate(
                bcast4 + [_make_mall(ownd_map[hh]).reshape(P, 16 * P)], axis=1
            )
        ).astype(NPBF)
        for hh in (0, 1)
    }

    in_maps = []
    for core in range(8):
        b, hh = core // 2, core % 2
        ownd = ownd_map[hh]
        own = np.concatenate([np.arange(blk * P, (blk + 1) * P) for blk in ownd])
        xb = x[b]  # (T, E)
        xT = np.ascontiguousarray(xb.T).astype(NPF8)
        # bv folded in: each head's output picks up exactly +bv after the
        # softmax division (weights sum to 1), so it lands here instead
        xTq = np.ascontiguousarray(
            xb[own].T + np.asarray(bv, np.float32)[:, None]
        ).astype(NPBF)
        xT8q = np.ascontiguousarray(xb[own].T).astype(NPF8)
        in_maps.append(
            {
                "xT": xT,
                "xTq": xTq,
                "xT8q": xT8q,
                "WqT": WqT,
                "WkT": WkT,
                "WvT": WvT,
                "WpT": WpT,
                "cF32": cF32,
                "cBF": cBF_map[hh],
            }
        )

    import os

    trace = bool(int(os.environ.get("MHSA_TRACE", "0")))
    res = run_bass_kernel_spmd(
        nc, in_maps, core_ids=list(range(8)), trace=trace,
        trace_cores=list(range(8)) if trace else None,
    )
    if trace and res.exec_time_ns is not None:
        print(f"HW exec time: {res.exec_time_ns} ns")
        if res.mean_exec_time_ns is not None:
            print(f"HW exec mean across cores: {res.mean_exec_time_ns:.0f} ns")
        kernel.last_exec_time_ns = res.exec_time_ns
        kernel.last_trace = res.instructions_and_trace

    out = np.empty((B, T, E), np.float32)
    for core in range(8):
        b, hh = core // 2, core % 2
        ownd = ownd_map[hh]
        y = res.results[core]["y"]  # (NQ, P, E) bf16
        for k, blk in enumerate(ownd):
            out[b, blk * P : (blk + 1) * P, :] = y[k].astype(np.float32)
    return out
